# revision 1
# baseline (speedup 1.0000x reference)
"""Trainium2 Bass kernel for AttentiveGnLConv (TransformerConv + GRU + GATv2 + GRU).

Sharding: nodes partitioned across 8 cores (6250/core, padded to 6272 = 49*128).
Edges routed to the core owning the *target* (dst) node, sorted by dst block
(128 dst nodes per block), padded so each block has the same chunk count on
every core (SPMD: one instruction stream for all 8 cores).

Per-core phases:
  A. node-feature tables: kv_table (k|v, all nodes, replicated compute, bf16),
     q_table (own nodes), skipT/lin0T (own nodes, feature-major, resident).
  B. conv1 edge pass: per dst-block, indirect-gather kv[src], q[dst]; e =
     edge_attr @ te_w on PE; logits/softmax-numerators via DVE; scatter-add to
     the block's 128 dst rows with a one-hot matmul accumulated in PSUM.
  C. node chain: elu -> GRU1 -> lin1 (feature-major, fp32r matmuls, weights
     stationary), xl/xr shards written node-major.
  D. AllGather xl shards -> full xl table.
  E. conv2 (GATv2) edge pass, same structure as B.
  F. node chain 2: elu -> GRU2 -> relu -> + g_sum -> output [64, 6272] f32
     (feature-major; host transposes/unpads/concats).
"""

import sys

for _p in ("/opt/trn_rl_repo",):
    if _p not in sys.path:
        sys.path.insert(0, _p)

import math
from contextlib import ExitStack

import numpy as np
import ml_dtypes

import concourse.bass as bass
import concourse.bacc as bacc
import concourse.tile as tile
from concourse import mybir
from concourse.bass import IndirectOffsetOnAxis
from concourse.masks import make_identity
from concourse.library_config import mlp as mlp_lib

FP = mybir.dt.float32
BF = mybir.dt.bfloat16
F32R = mybir.dt.float32r
I32 = mybir.dt.int32
AF = mybir.ActivationFunctionType
OP = mybir.AluOpType

NCORES = 8
IN, HC, HID, H, C, EDIM = 64, 128, 64, 4, 32, 16
INV_SQRT_C = 1.0 / math.sqrt(C)

# bias pack column indices
B_TS, B_L0, B_G1R, B_G1Z, B_G1IN, B_G1HN, B_LIN1, B_GL, B_GR, B_GB, \
    B_G2R, B_G2Z, B_G2IN, B_G2HN = range(14)
NB = 14


def _f32r(ap):
    return ap.bitcast(F32R)


def build_program(cfg):
    """Build the SPMD bass program. cfg: dict with
    npc (real nodes/core), npad (multiple of 128), K_blk (list, chunks per
    block), and chain chunk size ch (divides npad)."""
    npad = cfg["npad"]
    nblk = npad // 128
    nt = NCORES * npad
    half = nt // 2
    K_A = cfg["K_A"]
    K_B = cfg["K_B"]
    # groups of 2 blocks; slot order within group: A(b0) A(b1) B(b0) B(b1)
    ngrp = (nblk + 1) // 2
    grp_blocks = [tuple(b for b in (2 * g, 2 * g + 1) if b < nblk)
                  for g in range(ngrp)]
    # per-group chunk ranges
    grp_off = []     # start chunk (global slot-chunk index) of each group
    grp_KA = []      # total A chunks in group
    grp_K = []       # total chunks in group
    blk_chunks = {}  # block -> list of global chunk indices (A then B)
    pos = 0
    for g, bs in enumerate(grp_blocks):
        grp_off.append(pos)
        ka = sum(K_A[b] for b in bs)
        kb = sum(K_B[b] for b in bs)
        grp_KA.append(ka)
        grp_K.append(ka + kb)
        p = pos
        a_start = {}
        for b in bs:
            a_start[b] = p
            p += K_A[b]
        for b in bs:
            blk_chunks[b] = list(range(a_start[b], a_start[b] + K_A[b])) +                 list(range(p, p + K_B[b]))
            p += K_B[b]
        pos += ka + kb
    S = pos
    SA = sum(K_A)
    SB = sum(K_B)
    ch = cfg["ch"]
    nch = npad // ch
    assert nch * ch == npad

    nc = bacc.Bacc("TRN2", target_bir_lowering=False, debug=False,
                   num_devices=NCORES)

    # ---------------- DRAM parameters (inputs) ----------------
    def din(name, shape, dt):
        return nc.dram_tensor(name, shape, dt, kind="ExternalInput").ap()

    xt_full = din("xt_full", [IN, nt], BF)       # x^T, padded global layout
    xt_own = din("xt_own", [IN, npad], BF)       # per-core slice of xt_full
    idx_a = din("idx_a", [128, 8 * SA], mybir.dt.int16)   # src (A half)
    idx_b = din("idx_b", [128, 8 * SB], mybir.dt.int16)   # src-half (B half)
    idx_d = din("idx_d", [128, 8 * S], mybir.dt.int16)    # own-local dst
    dlocw = din("dlocw", [128, S], BF)           # dst-in-block (-1 pad)
    eat16 = din("eat16", [16, S * 128], BF)      # edge_attr^T per slot
    w_kv = din("w_kv", [IN, 2 * HC], BF)         # [tk_w | tv_w]
    w_q = din("w_q", [IN, HC], BF)
    w_te = din("w_te", [EDIM, HC], BF)
    w_ge = din("w_ge", [EDIM, HID], BF)
    w_sk = din("w_sk", [IN, HC], BF)             # ts_w
    w_l0 = din("w_l0", [IN, HC], BF)             # lin0_w
    w_g1 = din("w_g1", [HC, 6 * HC], BF)         # wi_r|wi_z|wi_n|wh_r|wh_z|wh_n
    w_lin1 = din("w_lin1", [HC, HID], BF)
    w_gl = din("w_gl", [HID, HID], BF)
    w_gr = din("w_gr", [HID, HID], BF)
    w_g2 = din("w_g2", [HID, 6 * HID], BF)
    biases = din("biases", [128, NB], FP)
    iota_in = din("iota_in", [128, 128], BF)     # row j value = j
    gatt_b = din("gatt_b", [128, HID], FP)       # g_att broadcast down parts
    kvb_b = din("kvb_b", [128, 2 * HC], FP)      # [tk_b|tv_b] broadcast
    tqb_b = din("tqb_b", [128, HC], FP)          # tq_b broadcast

    out_d = nc.dram_tensor("out", [HID, npad], FP, kind="ExternalOutput").ap()
    debug = cfg.get("debug", False)

    # ---------------- internal DRAM ----------------
    kv_table = nc.dram_tensor("kv_table", [nt, 2 * HC], BF).ap()
    q_table = nc.dram_tensor("q_table", [npad, HC], BF).ap()
    xl_shard = nc.dram_tensor("xl_shard", [npad, HID], FP).ap()
    xr_shard = nc.dram_tensor("xr_shard", [npad, HID], FP).ap()
    xl_table = nc.dram_tensor("xl_table", [nt, HID], FP,
                              addr_space="Shared").ap()

    if debug:
        dbg_kv = nc.dram_tensor("dbg_kv", [nt, 2 * HC], BF,
                                kind="ExternalOutput").ap()
        dbg_q = nc.dram_tensor("dbg_q", [npad, HC], BF,
                               kind="ExternalOutput").ap()
        dbg_c1 = nc.dram_tensor("dbg_c1", [HC, npad], BF,
                                kind="ExternalOutput").ap()
        dbg_sk = nc.dram_tensor("dbg_sk", [HC, npad], BF,
                                kind="ExternalOutput").ap()
        dbg_x10 = nc.dram_tensor("dbg_x10", [HC, npad], BF,
                                 kind="ExternalOutput").ap()
        dbg_x1f = nc.dram_tensor("dbg_x1f", [HID, npad], BF,
                                 kind="ExternalOutput").ap()
        dbg_xlt = nc.dram_tensor("dbg_xlt", [nt, HID], FP,
                                 kind="ExternalOutput").ap()
        dbg_c2 = nc.dram_tensor("dbg_c2", [HID, npad], BF,
                                kind="ExternalOutput").ap()
        K0 = grp_K[0]
        dbg_kvg = nc.dram_tensor("dbg_kvg", [128, K0 * 256], BF,
                                 kind="ExternalOutput").ap()
        dbg_qg = nc.dram_tensor("dbg_qg", [128, K0 * 128], BF,
                                kind="ExternalOutput").ap()
        dbg_esb = nc.dram_tensor("dbg_esb", [128, K0 * 128], BF,
                                 kind="ExternalOutput").ap()
        dbg_oh = nc.dram_tensor("dbg_oh", [128, K0 * 128], BF,
                                kind="ExternalOutput").ap()
        dbg_msgp = nc.dram_tensor("dbg_msgp", [128, K0 * 132], BF,
                                  kind="ExternalOutput").ap()

    with tile.TileContext(nc) as tc, ExitStack() as top:
        const = top.enter_context(tc.tile_pool(name="const", bufs=1))

        # constants into SBUF
        idn_f = const.tile([128, 128], FP)
        make_identity(nc, idn_f[:])
        idn_b = const.tile([128, 128], BF)
        make_identity(nc, idn_b[:])
        c_wkv = const.tile([IN, 2 * HC], BF)
        nc.sync.dma_start(c_wkv[:], w_kv[:])
        c_wq = const.tile([IN, HC], BF)
        nc.sync.dma_start(c_wq[:], w_q[:])
        c_wte = const.tile([EDIM, HC], BF)
        nc.sync.dma_start(c_wte[:], w_te[:])
        c_wge = const.tile([EDIM, HID], BF)
        nc.sync.dma_start(c_wge[:], w_ge[:])
        c_wsk = const.tile([IN, HC], BF)
        nc.sync.dma_start(c_wsk[:], w_sk[:])
        c_wl0 = const.tile([IN, HC], BF)
        nc.sync.dma_start(c_wl0[:], w_l0[:])
        c_wg1 = const.tile([HC, 6 * HC], BF)
        nc.sync.dma_start(c_wg1[:], w_g1[:])
        c_wlin1 = const.tile([HC, HID], BF)
        nc.sync.dma_start(c_wlin1[:], w_lin1[:])
        c_wgl = const.tile([HID, HID], BF)
        nc.sync.dma_start(c_wgl[:], w_gl[:])
        c_wgr = const.tile([HID, HID], BF)
        nc.sync.dma_start(c_wgr[:], w_gr[:])
        c_wg2 = const.tile([HID, 6 * HID], BF)
        nc.sync.dma_start(c_wg2[:], w_g2[:])
        c_bias = const.tile([128, NB], FP)
        nc.sync.dma_start(c_bias[:], biases[:])
        c_iota = const.tile([128, 128], BF)
        nc.sync.dma_start(c_iota[:], iota_in[:])
        c_gatt = const.tile([128, HID], FP)
        nc.sync.dma_start(c_gatt[:], gatt_b[:])
        c_kvb = const.tile([128, 2 * HC], FP)
        nc.sync.dma_start(c_kvb[:], kvb_b[:])
        c_tqb = const.tile([128, HC], FP)
        nc.sync.dma_start(c_tqb[:], tqb_b[:])

        def bcol(j, p=128):
            return c_bias[:p, j:j + 1]

        # edge metadata resident
        meta = top.enter_context(tc.tile_pool(name="meta", bufs=1))
        m_ia = meta.tile([128, 8 * SA], mybir.dt.int16)
        nc.sync.dma_start(m_ia[:], idx_a[:])
        m_ib = meta.tile([128, 8 * SB], mybir.dt.int16)
        nc.sync.dma_start(m_ib[:], idx_b[:])
        m_id = meta.tile([128, 8 * S], mybir.dt.int16)
        nc.sync.dma_start(m_id[:], idx_d[:])
        m_dloc = meta.tile([128, S], BF)
        nc.sync.dma_start(m_dloc[:], dlocw[:])
        nc.gpsimd.load_library(mlp_lib)

        # resident per-node feature-major tensors
        res = top.enter_context(tc.tile_pool(name="res", bufs=1))
        r_x1fT = res.tile([HID, npad], BF)     # x1 final (= g_sum)

        resA_cm = tc.tile_pool(name="resA", bufs=1)
        resA = resA_cm.__enter__()
        r_skT = resA.tile([HC, npad], BF)      # x@ts_w (+ts_b later)
        r_x10T = resA.tile([HC, npad], BF)     # lrelu(x@lin0_w + b)
        r_c1aT = resA.tile([HC, npad], BF)     # conv1 aggregation^T (normed)

        # ---------------- stage A: tables ----------------
        with tc.tile_pool(name="stA", bufs=2) as pA, \
                tc.tile_pool(name="psA", bufs=2, space="PSUM") as psA:
            ntile_g = nt // 128
            assert ntile_g % 8 == 0
            for s8 in range(ntile_g // 8):      # 8 node-tiles per iteration
                xa = pA.tile([IN, 8 * 128], BF, tag="xa")
                nc.sync.dma_start(xa[:], xt_full[:, s8 * 1024:(s8 + 1) * 1024])
                stg = pA.tile([128, 8 * 256], BF, tag="kvstg")
                for j2 in range(4):             # psum pairs
                    ps = psA.tile([128, 512], FP, tag="pskv")
                    for j in range(2):
                        t = j2 * 2 + j
                        nc.tensor.matmul(ps[:, j * 256:(j + 1) * 256],
                                         lhsT=xa[:, t * 128:(t + 1) * 128],
                                         rhs=c_wkv[:], start=True, stop=True)
                    for j in range(2):
                        t = j2 * 2 + j
                        nc.vector.tensor_tensor(
                            out=stg[:, t * 256:(t + 1) * 256],
                            in0=ps[:, j * 256:(j + 1) * 256],
                            in1=c_kvb[:], op=OP.add)
                nc.sync.dma_start(
                    kv_table[s8 * 1024:(s8 + 1) * 1024, :].rearrange(
                        "(j p) f -> p j f", p=128),
                    stg[:].rearrange("p (j f) -> p j f", f=256))

            for t in range(nblk):
                xo = pA.tile([IN, 128], BF, tag="xo")
                nc.sync.dma_start(xo[:], xt_own[:, t * 128:(t + 1) * 128])
                psq = psA.tile([128, 128], FP, tag="psq")
                nc.tensor.matmul(psq[:], lhsT=xo[:], rhs=c_wq[:],
                                 start=True, stop=True)
                qs = pA.tile([128, HC], BF, tag="qstg")
                nc.vector.tensor_tensor(out=qs[:], in0=psq[:], in1=c_tqb[:],
                                        op=OP.add)
                nc.scalar.dma_start(q_table[t * 128:(t + 1) * 128, :], qs[:])
                # feature-major skip / lin0 for own nodes
                pssl = psA.tile([128, 256], FP, tag="pssl")
                nc.tensor.matmul(pssl[:, 0:128], lhsT=c_wsk[:], rhs=xo[:],
                                 start=True, stop=True)
                nc.tensor.matmul(pssl[:, 128:256], lhsT=c_wl0[:], rhs=xo[:],
                                 start=True, stop=True)
                cols = slice(t * 128, (t + 1) * 128)
                nc.scalar.activation(r_skT[:, cols], pssl[:, 0:128], AF.Copy)
                tl0 = pA.tile([128, 128], FP, tag="tl0")
                nc.vector.tensor_scalar_add(tl0[:], pssl[:, 128:256],
                                            bcol(B_L0))
                nc.vector.scalar_tensor_tensor(
                    out=r_x10T[:, cols], in0=tl0[:], scalar=0.01, in1=tl0[:],
                    op0=OP.mult, op1=OP.max)

        # ---------------- stage B: conv1 edges + stage C chain ----------------
        with tc.tile_pool(name="stB", bufs=2) as pB, \
                tc.tile_pool(name="stBw", bufs=1) as pBw, \
                tc.tile_pool(name="psB", bufs=2, space="PSUM") as psB, \
                tc.tile_pool(name="psBa", bufs=1, space="PSUM") as psBa, \
                tc.tile_pool(name="stBs", bufs=1) as pBs:
            for g in range(ngrp):
                Kg = grp_K[g]
                KAg = grp_KA[g]
                KBg = Kg - KAg
                c0 = grp_off[g]
                kvg = pB.tile([128, Kg * 256], BF, tag="kvg")
                kv3 = kvg[:].rearrange("p (k f) -> p k f", f=256)
                nc.gpsimd.dma_gather(
                    kv3[:, 0:KAg, :], kv_table[0:half, :],
                    m_ia[:, 8 * sum(grp_KA[:g]):8 * (sum(grp_KA[:g]) + KAg)],
                    KAg * 128, KAg * 128, 256, single_packet=False)
                b0s = 8 * (sum(grp_K[:g]) - sum(grp_KA[:g]))
                nc.gpsimd.dma_gather(
                    kv3[:, KAg:Kg, :], kv_table[half:nt, :],
                    m_ib[:, b0s:b0s + 8 * KBg],
                    KBg * 128, KBg * 128, 256, single_packet=False)
                qg = pB.tile([128, Kg * 128], BF, tag="qg")
                nc.gpsimd.dma_gather(
                    qg[:].rearrange("p (k f) -> p k f", f=128),
                    q_table[:],
                    m_id[:, 8 * c0:8 * (c0 + Kg)],
                    Kg * 128, Kg * 128, 128, single_packet=False)
                eat = pB.tile([16, Kg * 128], BF, tag="eat")
                nc.scalar.dma_start(eat[:],
                                    eat16[:, c0 * 128:(c0 + Kg) * 128])
                e_sb = pBw.tile([128, Kg * 128], BF, tag="esb")
                for q4 in range((Kg + 3) // 4):
                    kk = min(4, Kg - q4 * 4)
                    pse = psB.tile([128, 512], FP, tag="pse")
                    for cc in range(kk):
                        ca = q4 * 4 + cc
                        nc.tensor.matmul(
                            pse[:, cc * 128:(cc + 1) * 128],
                            lhsT=eat[:, ca * 128:(ca + 1) * 128],
                            rhs=c_wte[:], start=True, stop=True)
                    nc.scalar.activation(
                        e_sb[:, q4 * 512:q4 * 512 + kk * 128],
                        pse[:, :kk * 128], AF.Copy)
                keye = pBw.tile([128, Kg * 128], BF, tag="keye")
                k3 = keye[:].rearrange("p (k f) -> p k f", f=128)
                e3 = e_sb[:].rearrange("p (k f) -> p k f", f=128)
                nc.vector.tensor_tensor(out=k3, in0=kv3[:, :, 0:128], in1=e3,
                                        op=OP.add)
                prod = pBw.tile([128, Kg * 128], BF, tag="prod")
                nc.vector.tensor_tensor(
                    out=prod[:], in0=keye[:], in1=qg[:], op=OP.mult)
                lgt = pBw.tile([128, Kg * 4], FP, tag="lgt")
                nc.vector.tensor_reduce(
                    lgt[:].rearrange("p (k h) -> p k h", h=4),
                    prod[:].rearrange("p (k h c) -> p k h c", h=4, c=32),
                    axis=mybir.AxisListType.X, op=OP.add)
                msgp = pBw.tile([128, Kg * 132], BF, tag="msgp")
                mp3 = msgp[:].rearrange("p (k f) -> p k f", f=132)
                nc.scalar.activation(
                    mp3[:, :, 128:132],
                    lgt[:].rearrange("p (k h) -> p k h", h=4),
                    AF.Exp, scale=INV_SQRT_C)
                msgv = pBw.tile([128, Kg * 128], BF, tag="keye")
                mv3 = msgv[:].rearrange("p (k f) -> p k f", f=128)
                nc.vector.tensor_tensor(out=mv3, in0=kv3[:, :, 128:256],
                                        in1=e3, op=OP.add)
                nc.vector.tensor_tensor(
                    out=mp3[:, :, 0:128].rearrange("p k (h c) -> p k h c",
                                                   c=32),
                    in0=msgv[:].rearrange("p (k h c) -> p k h c", h=4, c=32),
                    in1=mp3[:, :, 128:132].unsqueeze(3).to_broadcast(
                        [128, Kg, 4, 32]),
                    op=OP.mult)
                oh = pBw.tile([128, Kg * 128], BF, tag="prod")
                nc.vector.tensor_tensor(
                    out=oh[:].rearrange("p (k f) -> p k f", f=128),
                    in0=c_iota[:].unsqueeze(1).to_broadcast([128, Kg, 128]),
                    in1=m_dloc[:, c0:c0 + Kg].unsqueeze(2).to_broadcast(
                        [128, Kg, 128]),
                    op=OP.is_equal)
                if debug and g == 0:
                    nc.sync.dma_start(dbg_kvg[:], kvg[:, :dbg_kvg.shape[1]])
                    nc.sync.dma_start(dbg_qg[:], qg[:, :dbg_qg.shape[1]])
                    nc.sync.dma_start(dbg_esb[:], e_sb[:, :dbg_esb.shape[1]])
                    nc.sync.dma_start(dbg_oh[:], oh[:, :dbg_oh.shape[1]])
                    nc.sync.dma_start(dbg_msgp[:],
                                      msgp[:, :dbg_msgp.shape[1]])
                for b in grp_blocks[g]:
                    chs = [cc - c0 for cc in blk_chunks[b]]
                    pagg = psBa.tile([128, 132], FP, tag="agg")
                    for ci, cc in enumerate(chs):
                        nc.tensor.matmul(
                            pagg[:],
                            lhsT=oh[:, cc * 128:(cc + 1) * 128],
                            rhs=msgp[:, cc * 132:(cc + 1) * 132],
                            start=(ci == 0), stop=(ci == len(chs) - 1))
                    den = pBs.tile([128, 4], FP, tag="den")
                    nc.vector.tensor_scalar_add(den[:], pagg[:, 128:132],
                                                1e-16)
                    rec = pBs.tile([128, 4], FP, tag="rec")
                    nc.vector.reciprocal(rec[:], den[:])
                    aggn = pBs.tile([128, 128], BF, tag="aggn")
                    for h in range(H):
                        nc.vector.tensor_scalar_mul(
                            aggn[:, h * 32:(h + 1) * 32],
                            pagg[:, h * 32:(h + 1) * 32], rec[:, h:h + 1])
                    pst = psBa.tile([128, 128], BF, tag="ptr1")
                    nc.tensor.transpose(pst[:], aggn[:], idn_b[:])
                    nc.scalar.activation(r_c1aT[:, b * 128:(b + 1) * 128],
                                         pst[:], AF.Copy)

            # ---- stage C: chain 1 (feature-major, chunks of ch) ----
            for t in range(nch):
                cols = slice(t * ch, (t + 1) * ch)
                c1 = pBs.tile([128, ch], FP, tag="c1")
                nc.vector.tensor_tensor(out=c1[:], in0=r_c1aT[:, cols],
                                        in1=r_skT[:, cols], op=OP.add)
                mn = pBs.tile([128, ch], FP, tag="mn")
                nc.vector.tensor_scalar(out=mn[:], in0=c1[:],
                                        scalar1=bcol(B_TS), scalar2=0.0,
                                        op0=OP.add, op1=OP.min)
                mx = pBs.tile([128, ch], FP, tag="mx")
                nc.vector.tensor_scalar(out=mx[:], in0=c1[:],
                                        scalar1=bcol(B_TS), scalar2=0.0,
                                        op0=OP.add, op1=OP.max)
                em = pBs.tile([128, ch], FP, tag="em")
                nc.scalar.activation(em[:], mn[:], AF.Exp)
                hh = pBs.tile([128, ch], BF, tag="hh")
                nc.vector.scalar_tensor_tensor(
                    out=hh[:], in0=em[:], scalar=-1.0, in1=mx[:],
                    op0=OP.add, op1=OP.add)
                pr = psB.tile([128, ch], FP, tag="pg")
                nc.tensor.matmul(pr[:], lhsT=c_wg1[:, 0:128],
                                 rhs=hh[:], start=True, stop=False)
                nc.tensor.matmul(pr[:], lhsT=c_wg1[:, 384:512],
                                 rhs=r_x10T[:, cols],
                                 start=False, stop=True)
                pz = psB.tile([128, ch], FP, tag="pg")
                nc.tensor.matmul(pz[:], lhsT=c_wg1[:, 128:256],
                                 rhs=hh[:], start=True, stop=False)
                nc.tensor.matmul(pz[:], lhsT=c_wg1[:, 512:640],
                                 rhs=r_x10T[:, cols],
                                 start=False, stop=True)
                pin = psB.tile([128, ch], FP, tag="pg")
                nc.tensor.matmul(pin[:], lhsT=c_wg1[:, 256:384],
                                 rhs=hh[:], start=True, stop=True)
                phn = psB.tile([128, ch], FP, tag="pg")
                nc.tensor.matmul(phn[:], lhsT=c_wg1[:, 640:768],
                                 rhs=r_x10T[:, cols],
                                 start=True, stop=True)
                r_ = pBs.tile([128, ch], FP, tag="r_")
                nc.scalar.activation(r_[:], pr[:], AF.Sigmoid,
                                     bias=bcol(B_G1R))
                z_ = pBs.tile([128, ch], FP, tag="z_")
                nc.scalar.activation(z_[:], pz[:], AF.Sigmoid,
                                     bias=bcol(B_G1Z))
                thn = pBs.tile([128, ch], FP, tag="thn")
                nc.vector.scalar_tensor_tensor(
                    out=thn[:], in0=phn[:], scalar=bcol(B_G1HN), in1=r_[:],
                    op0=OP.add, op1=OP.mult)
                pre = pBs.tile([128, ch], FP, tag="pre")
                nc.vector.tensor_tensor(out=pre[:], in0=pin[:], in1=thn[:],
                                        op=OP.add)
                n_ = pBs.tile([128, ch], FP, tag="n_")
                nc.scalar.activation(n_[:], pre[:], AF.Tanh,
                                     bias=bcol(B_G1IN))
                d_ = pBs.tile([128, ch], FP, tag="d_")
                nc.vector.tensor_tensor(out=d_[:], in0=r_x10T[:, cols],
                                        in1=n_[:], op=OP.subtract)
                zd = pBs.tile([128, ch], FP, tag="zd")
                nc.vector.tensor_tensor(out=zd[:], in0=z_[:], in1=d_[:],
                                        op=OP.mult)
                x1g = pBs.tile([128, ch], FP, tag="x1g")
                nc.vector.tensor_tensor(out=x1g[:], in0=n_[:], in1=zd[:],
                                        op=OP.add)
                x1r = pBs.tile([128, ch], BF, tag="x1r")
                nc.scalar.activation(x1r[:], x1g[:], AF.Relu)
                psl = psBa.tile([HID, ch], FP, tag="psx")
                nc.tensor.matmul(psl[:], lhsT=c_wlin1[:],
                                 rhs=x1r[:], start=True, stop=True)
                tl1 = pBs.tile([HID, ch], FP, tag="tl1")
                nc.vector.tensor_scalar_add(tl1[:], psl[:],
                                            bcol(B_LIN1, HID))
                nc.vector.scalar_tensor_tensor(
                    out=r_x1fT[:, cols], in0=tl1[:], scalar=0.01, in1=tl1[:],
                    op0=OP.mult, op1=OP.max)

            # ---- xl / xr shards (node-major), per 2 blocks ----
            for b2 in range(nblk // 2 + (nblk % 2)):
                b = b2 * 2
                nb2 = min(2, nblk - b)
                w = nb2 * 128
                cols = slice(b * 128, b * 128 + w)
                psx = psBa.tile([HID, 512], FP, tag="psx")
                nc.tensor.matmul(psx[:, 0:w], lhsT=c_wgl[:],
                                 rhs=r_x1fT[:, cols],
                                 start=True, stop=True)
                nc.tensor.matmul(psx[:, 256:256 + w], lhsT=c_wgr[:],
                                 rhs=r_x1fT[:, cols],
                                 start=True, stop=True)
                xls = pBs.tile([HID, 256], FP, tag="xls")
                nc.vector.tensor_scalar_add(xls[:, :w], psx[:, 0:w],
                                            bcol(B_GL, HID))
                xrs = pBs.tile([HID, 256], FP, tag="xrs")
                nc.vector.tensor_scalar_add(xrs[:, :w], psx[:, 256:256 + w],
                                            bcol(B_GR, HID))
                ptx = psBa.tile([128, 256], FP, tag="ptr2")
                for j in range(nb2):
                    nc.tensor.transpose(ptx[:, j * 64:(j + 1) * 64],
                                        xls[:, j * 128:(j + 1) * 128],
                                        idn_f[:HID, :HID])
                    nc.tensor.transpose(ptx[:, 128 + j * 64:128 + (j + 1) * 64],
                                        xrs[:, j * 128:(j + 1) * 128],
                                        idn_f[:HID, :HID])
                stx = pBs.tile([128, 256], FP, tag="stx")
                nc.scalar.activation(stx[:, 0:nb2 * 64],
                                     ptx[:, 0:nb2 * 64], AF.Copy)
                nc.scalar.activation(stx[:, 128:128 + nb2 * 64],
                                     ptx[:, 128:128 + nb2 * 64], AF.Copy)
                nc.scalar.dma_start(
                    xl_shard[b * 128:b * 128 + w, :].rearrange(
                        "(j p) f -> p j f", p=128),
                    stx[:].rearrange("p (j f) -> p j f", f=64)[:, 0:nb2])
                nc.scalar.dma_start(
                    xr_shard[b * 128:b * 128 + w, :].rearrange(
                        "(j p) f -> p j f", p=128),
                    stx[:].rearrange("p (j f) -> p j f", f=64)[:, 2:2 + nb2])

        if debug:
            nc.sync.dma_start(dbg_c1[:], r_c1aT[:])
            nc.sync.dma_start(dbg_sk[:], r_skT[:])
            nc.sync.dma_start(dbg_x10[:], r_x10T[:])
        resA_cm.__exit__(None, None, None)

        # ---------------- stage D: AllGather xl ----------------
        nc.gpsimd.collective_compute(
            "AllGather", OP.bypass,
            ins=[xl_shard[:]], outs=[xl_table[:]],
            replica_groups=[list(range(NCORES))])

        # ---------------- stage E: conv2 edges + chain 2 ----------------
        with tc.tile_pool(name="stE", bufs=2) as pE, \
                tc.tile_pool(name="stEw", bufs=1) as pEw, \
                tc.tile_pool(name="psE", bufs=2, space="PSUM") as psE, \
                tc.tile_pool(name="psEa", bufs=1, space="PSUM") as psEa, \
                tc.tile_pool(name="stEs", bufs=1) as pEs:
            r_c2aT = pEs.tile([HID, npad], BF, tag="c2aT")
            for g in range(ngrp):
                Kg = grp_K[g]
                KAg = grp_KA[g]
                KBg = Kg - KAg
                c0 = grp_off[g]
                xlg = pE.tile([128, Kg * 64], FP, tag="xlg")
                xl3 = xlg[:].rearrange("p (k f) -> p k f", f=64)
                nc.gpsimd.dma_gather(
                    xl3[:, 0:KAg, :], xl_table[0:half, :],
                    m_ia[:, 8 * sum(grp_KA[:g]):8 * (sum(grp_KA[:g]) + KAg)],
                    KAg * 128, KAg * 128, 64, single_packet=False)
                b0s = 8 * (sum(grp_K[:g]) - sum(grp_KA[:g]))
                nc.gpsimd.dma_gather(
                    xl3[:, KAg:Kg, :], xl_table[half:nt, :],
                    m_ib[:, b0s:b0s + 8 * KBg],
                    KBg * 128, KBg * 128, 64, single_packet=False)
                xrg = pE.tile([128, Kg * 64], FP, tag="xrg")
                nc.gpsimd.dma_gather(
                    xrg[:].rearrange("p (k f) -> p k f", f=64),
                    xr_shard[:],
                    m_id[:, 8 * c0:8 * (c0 + Kg)],
                    Kg * 128, Kg * 128, 64, single_packet=False)
                zsum = pEw.tile([128, Kg * 64], FP, tag="zsum")
                nc.vector.tensor_tensor(out=zsum[:], in0=xlg[:], in1=xrg[:],
                                        op=OP.add)
                eat = pE.tile([16, Kg * 128], BF, tag="eat")
                nc.scalar.dma_start(eat[:],
                                    eat16[:, c0 * 128:(c0 + Kg) * 128])
                zed = pEw.tile([128, Kg * 64], FP, tag="zed")
                for q4 in range((Kg + 3) // 4):
                    kk = min(4, Kg - q4 * 4)
                    pse2 = psE.tile([128, 256], FP, tag="pse2")
                    for cc in range(kk):
                        ca = q4 * 4 + cc
                        nc.tensor.matmul(
                            pse2[:, cc * 64:(cc + 1) * 64],
                            lhsT=eat[:, ca * 128:(ca + 1) * 128],
                            rhs=c_wge[:], start=True, stop=True)
                    nc.vector.tensor_tensor(
                        out=zed[:, q4 * 256:q4 * 256 + kk * 64],
                        in0=pse2[:, :kk * 64],
                        in1=zsum[:, q4 * 256:q4 * 256 + kk * 64], op=OP.add)
                zl = pEw.tile([128, Kg * 64], FP, tag="zl")
                nc.vector.scalar_tensor_tensor(
                    out=zl[:], in0=zed[:], scalar=0.2, in1=zed[:],
                    op0=OP.mult, op1=OP.max)
                pr2 = pEw.tile([128, Kg * 64], FP, tag="pr2")
                nc.vector.tensor_tensor(
                    out=pr2[:].rearrange("p (k f) -> p k f", f=64),
                    in0=zl[:].rearrange("p (k f) -> p k f", f=64),
                    in1=c_gatt[:, :64].unsqueeze(1).to_broadcast(
                        [128, Kg, 64]),
                    op=OP.mult)
                lg2 = pEw.tile([128, Kg], FP, tag="lg2")
                nc.vector.tensor_reduce(
                    lg2[:].unsqueeze(2),
                    pr2[:].rearrange("p (k f) -> p k f", f=64),
                    axis=mybir.AxisListType.X, op=OP.add)
                mq2 = pEw.tile([128, Kg * 65], BF, tag="mq2")
                m23 = mq2[:].rearrange("p (k f) -> p k f", f=65)
                nc.scalar.activation(m23[:, :, 64:65],
                                     lg2[:].unsqueeze(2), AF.Exp)
                nc.vector.tensor_tensor(
                    out=m23[:, :, 0:64],
                    in0=xlg[:].rearrange("p (k f) -> p k f", f=64),
                    in1=m23[:, :, 64:65].to_broadcast([128, Kg, 64]),
                    op=OP.mult)
                oh2 = pEw.tile([128, Kg * 128], BF, tag="oh2")
                nc.vector.tensor_tensor(
                    out=oh2[:].rearrange("p (k f) -> p k f", f=128),
                    in0=c_iota[:].unsqueeze(1).to_broadcast([128, Kg, 128]),
                    in1=m_dloc[:, c0:c0 + Kg].unsqueeze(2).to_broadcast(
                        [128, Kg, 128]),
                    op=OP.is_equal)
                for b in grp_blocks[g]:
                    chs = [cc - c0 for cc in blk_chunks[b]]
                    pagg2 = psEa.tile([128, 65], FP, tag="agg2")
                    for ci, cc in enumerate(chs):
                        nc.tensor.matmul(
                            pagg2[:],
                            lhsT=oh2[:, cc * 128:(cc + 1) * 128],
                            rhs=mq2[:, cc * 65:(cc + 1) * 65],
                            start=(ci == 0), stop=(ci == len(chs) - 1))
                    den2 = pEs.tile([128, 1], FP, tag="den2")
                    nc.vector.tensor_scalar_add(den2[:], pagg2[:, 64:65],
                                                1e-16)
                    rec2 = pEs.tile([128, 1], FP, tag="rec2")
                    nc.vector.reciprocal(rec2[:], den2[:])
                    ag2 = pEs.tile([128, 64], BF, tag="ag2")
                    nc.vector.tensor_scalar_mul(ag2[:], pagg2[:, 0:64],
                                                rec2[:])
                    pst2 = psEa.tile([HID, 128], BF, tag="ptr3")
                    nc.tensor.transpose(pst2[:], ag2[:], idn_b[:])
                    nc.scalar.activation(r_c2aT[:, b * 128:(b + 1) * 128],
                                         pst2[:], AF.Copy)
            if debug:
                nc.sync.dma_start(dbg_c2[:], r_c2aT[:])

            # ---- chain 2 ----
            for t in range(nch):
                cols = slice(t * ch, (t + 1) * ch)
                mn2 = pEs.tile([HID, ch], FP, tag="mn2")
                nc.vector.tensor_scalar(out=mn2[:], in0=r_c2aT[:HID, cols],
                                        scalar1=bcol(B_GB, HID), scalar2=0.0,
                                        op0=OP.add, op1=OP.min)
                mx2 = pEs.tile([HID, ch], FP, tag="mx2")
                nc.vector.tensor_scalar(out=mx2[:], in0=r_c2aT[:HID, cols],
                                        scalar1=bcol(B_GB, HID), scalar2=0.0,
                                        op0=OP.add, op1=OP.max)
                em2 = pEs.tile([HID, ch], FP, tag="em2")
                nc.scalar.activation(em2[:], mn2[:], AF.Exp)
                h2 = pEs.tile([HID, ch], BF, tag="h2")
                nc.vector.scalar_tensor_tensor(
                    out=h2[:], in0=em2[:], scalar=-1.0, in1=mx2[:],
                    op0=OP.add, op1=OP.add)
                pr = psE.tile([HID, ch], FP, tag="pg2")
                nc.tensor.matmul(pr[:], lhsT=c_wg2[:, 0:64],
                                 rhs=h2[:], start=True, stop=False)
                nc.tensor.matmul(pr[:], lhsT=c_wg2[:, 192:256],
                                 rhs=r_x1fT[:, cols],
                                 start=False, stop=True)
                pz = psE.tile([HID, ch], FP, tag="pg2")
                nc.tensor.matmul(pz[:], lhsT=c_wg2[:, 64:128],
                                 rhs=h2[:], start=True, stop=False)
                nc.tensor.matmul(pz[:], lhsT=c_wg2[:, 256:320],
                                 rhs=r_x1fT[:, cols],
                                 start=False, stop=True)
                pin = psE.tile([HID, ch], FP, tag="pg2")
                nc.tensor.matmul(pin[:], lhsT=c_wg2[:, 128:192],
                                 rhs=h2[:], start=True, stop=True)
                phn = psE.tile([HID, ch], FP, tag="pg2")
                nc.tensor.matmul(phn[:], lhsT=c_wg2[:, 320:384],
                                 rhs=r_x1fT[:, cols],
                                 start=True, stop=True)
                r2 = pEs.tile([HID, ch], FP, tag="r2")
                nc.scalar.activation(r2[:], pr[:], AF.Sigmoid,
                                     bias=bcol(B_G2R, HID))
                z2 = pEs.tile([HID, ch], FP, tag="z2")
                nc.scalar.activation(z2[:], pz[:], AF.Sigmoid,
                                     bias=bcol(B_G2Z, HID))
                th2 = pEs.tile([HID, ch], FP, tag="th2")
                nc.vector.scalar_tensor_tensor(
                    out=th2[:], in0=phn[:], scalar=bcol(B_G2HN, HID),
                    in1=r2[:], op0=OP.add, op1=OP.mult)
                pre2 = pEs.tile([HID, ch], FP, tag="pre2")
                nc.vector.tensor_tensor(out=pre2[:], in0=pin[:], in1=th2[:],
                                        op=OP.add)
                n2 = pEs.tile([HID, ch], FP, tag="n2")
                nc.scalar.activation(n2[:], pre2[:], AF.Tanh,
                                     bias=bcol(B_G2IN, HID))
                d2 = pEs.tile([HID, ch], FP, tag="d2")
                nc.vector.tensor_tensor(out=d2[:], in0=r_x1fT[:, cols],
                                        in1=n2[:], op=OP.subtract)
                zd2 = pEs.tile([HID, ch], FP, tag="zd2")
                nc.vector.tensor_tensor(out=zd2[:], in0=z2[:], in1=d2[:],
                                        op=OP.mult)
                x2g = pEs.tile([HID, ch], FP, tag="x2g")
                nc.vector.tensor_tensor(out=x2g[:], in0=n2[:], in1=zd2[:],
                                        op=OP.add)
                x2r = pEs.tile([HID, ch], FP, tag="x2r")
                nc.scalar.activation(x2r[:], x2g[:], AF.Relu)
                oo = pEs.tile([HID, ch], FP, tag="oo")
                nc.vector.tensor_tensor(out=oo[:], in0=x2r[:],
                                        in1=r_x1fT[:, cols], op=OP.add)
                nc.sync.dma_start(out_d[:, cols], oo[:])

        if debug:
            nc.sync.dma_start(dbg_kv[:], kv_table[:])
            nc.sync.dma_start(dbg_q[:], q_table[:])
            nc.sync.dma_start(dbg_x1f[:], r_x1fT[:])
            nc.sync.dma_start(dbg_xlt[:], xl_table[:])

    nc.compile()
    return nc


# ======================= host-side data prep =======================

def prep_inputs(inputs, npc, npad):
    """Build per-core in_maps + cfg from the full problem inputs."""
    x = np.asarray(inputs["x"], np.float32)
    ea = np.asarray(inputs["edge_attr"], np.float32)
    ei = np.asarray(inputs["edge_index"], np.int64)
    n = x.shape[0]
    assert n == NCORES * npc
    nblk = npad // 128
    nt = NCORES * npad
    half = nt // 2
    src, dst = ei[0], ei[1]
    dcore = dst // npc
    dloc_in_core = dst - dcore * npc
    blk = dloc_in_core // 128
    dloc = dloc_in_core % 128
    srcp = (src // npc) * npad + (src % npc)   # padded global id
    isA = srcp < half

    order = np.lexsort((blk, dcore))
    keys = dcore[order] * nblk + blk[order]
    bounds = np.searchsorted(keys, np.arange(NCORES * nblk + 1))
    el_A, el_B = [], []
    KA_cb = np.zeros((NCORES, nblk), np.int64)
    KB_cb = np.zeros((NCORES, nblk), np.int64)
    for c in range(NCORES):
        for b in range(nblk):
            j = c * nblk + b
            el = order[bounds[j]:bounds[j + 1]]
            a = el[isA[el]]
            bb = el[~isA[el]]
            el_A.append(a)
            el_B.append(bb)
            KA_cb[c, b] = (len(a) + 127) // 128
            KB_cb[c, b] = (len(bb) + 127) // 128
    K_A = [int(max(1, KA_cb[:, b].max())) for b in range(nblk)]
    K_B = [int(max(1, KB_cb[:, b].max())) for b in range(nblk)]
    ngrp = (nblk + 1) // 2
    grp_blocks = [tuple(b for b in (2 * g, 2 * g + 1) if b < nblk)
                  for g in range(ngrp)]
    S = sum(K_A) + sum(K_B)
    SA = sum(K_A)
    SB = sum(K_B)

    def tile16(a):
        w = a.reshape(-1, 16).T
        return np.ascontiguousarray(np.tile(w, (8, 1)))

    per_core = []
    for c in range(NCORES):
        dlocs = np.full(S * 128, -1.0, np.float32)
        downs = np.zeros(S * 128, np.int64)
        eas = np.zeros((S * 128, EDIM), np.float32)
        ia = np.zeros(SA * 128, np.int64)
        ib = np.zeros(SB * 128, np.int64)
        pos = apos = bpos = 0
        for g in range(ngrp):
            for b in grp_blocks[g]:           # A slots
                el = el_A[c * nblk + b]
                ne = len(el)
                dlocs[pos:pos + ne] = dloc[el]
                downs[pos:pos + ne] = b * 128 + dloc[el]
                eas[pos:pos + ne] = ea[el]
                ia[apos:apos + ne] = srcp[el]
                pos += K_A[b] * 128
                apos += K_A[b] * 128
            for b in grp_blocks[g]:           # B slots
                el = el_B[c * nblk + b]
                ne = len(el)
                dlocs[pos:pos + ne] = dloc[el]
                downs[pos:pos + ne] = b * 128 + dloc[el]
                eas[pos:pos + ne] = ea[el]
                ib[bpos:bpos + ne] = srcp[el] - half
                pos += K_B[b] * 128
                bpos += K_B[b] * 128
        assert pos == S * 128 and apos == SA * 128 and bpos == SB * 128

        def wrap(a):
            return np.ascontiguousarray(a.reshape(-1, 128).T)

        per_core.append(dict(
            idx_a=tile16(ia.astype(np.int16)),
            idx_b=tile16(ib.astype(np.int16)),
            idx_d=tile16(downs.astype(np.int16)),
            dlocw=wrap(dlocs).astype(ml_dtypes.bfloat16),
            eat16=np.ascontiguousarray(eas.T).astype(ml_dtypes.bfloat16)))

    # shared tensors
    xt = np.zeros((IN, nt), np.float32)
    for c in range(NCORES):
        xt[:, c * npad:c * npad + npc] = x[c * npc:(c + 1) * npc].T
    bf = ml_dtypes.bfloat16
    shared = dict(
        xt_full=xt.astype(bf),
        w_kv=np.concatenate([inputs["tk_w"], inputs["tv_w"]],
                            axis=1).astype(bf),
        w_q=np.asarray(inputs["tq_w"]).astype(bf),
        w_te=np.asarray(inputs["te_w"]).astype(bf),
        w_ge=np.asarray(inputs["ge_w"]).astype(bf),
        w_sk=np.asarray(inputs["ts_w"]).astype(bf),
        w_l0=np.asarray(inputs["lin0_w"]).astype(bf),
        w_g1=np.concatenate([inputs["g1_wi"], inputs["g1_wh"]],
                            axis=1).astype(bf),
        w_lin1=np.asarray(inputs["lin1_w"]).astype(bf),
        w_gl=np.asarray(inputs["gl_w"]).astype(bf),
        w_gr=np.asarray(inputs["gr_w"]).astype(bf),
        w_g2=np.concatenate([inputs["g2_wi"], inputs["g2_wh"]],
                            axis=1).astype(bf),
        iota_in=np.tile(np.arange(128, dtype=np.float32), (128, 1)).astype(bf),
        gatt_b=np.tile(np.asarray(inputs["g_att"], np.float32), (128, 1)),
        kvb_b=np.tile(np.concatenate([inputs["tk_b"], inputs["tv_b"]]),
                      (128, 1)).astype(np.float32),
        tqb_b=np.tile(np.asarray(inputs["tq_b"], np.float32), (128, 1)),
    )
    bias = np.zeros((128, NB), np.float32)

    def put(j, v):
        v = np.asarray(v, np.float32)
        bias[:len(v), j] = v

    put(B_TS, inputs["ts_b"])
    put(B_L0, inputs["lin0_b"])
    g1bi, g1bh = np.asarray(inputs["g1_bi"]), np.asarray(inputs["g1_bh"])
    put(B_G1R, g1bi[0:128] + g1bh[0:128])
    put(B_G1Z, g1bi[128:256] + g1bh[128:256])
    put(B_G1IN, g1bi[256:384])
    put(B_G1HN, g1bh[256:384])
    put(B_LIN1, inputs["lin1_b"])
    put(B_GL, inputs["gl_b"])
    put(B_GR, inputs["gr_b"])
    put(B_GB, inputs["g_bias"])
    g2bi, g2bh = np.asarray(inputs["g2_bi"]), np.asarray(inputs["g2_bh"])
    put(B_G2R, g2bi[0:64] + g2bh[0:64])
    put(B_G2Z, g2bi[64:128] + g2bh[64:128])
    put(B_G2IN, g2bi[128:192])
    put(B_G2HN, g2bh[128:192])
    shared["biases"] = bias

    in_maps = []
    for c in range(NCORES):
        m = dict(shared)
        m["xt_own"] = np.ascontiguousarray(
            xt[:, c * npad:(c + 1) * npad]).astype(bf)
        m.update(per_core[c])
        in_maps.append(m)
    cfg = dict(npc=npc, npad=npad, K_A=K_A, K_B=K_B,
               ch=448 if npad % 448 == 0 else 128)
    return in_maps, cfg


def run(inputs, npc, npad, trace=False):
    in_maps, cfg = prep_inputs(inputs, npc, npad)
    nc = build_program(cfg)
    from concourse.bass_utils import run_bass_kernel_spmd
    res = run_bass_kernel_spmd(nc, in_maps, list(range(NCORES)), trace=trace)
    outs = []
    for c in range(NCORES):
        o = res.results[c]["out"]          # [HID, npad]
        outs.append(o[:, :npc].T)
    full = np.concatenate(outs, axis=0).astype(np.float32)
    return full, res


def kernel(**inputs) -> np.ndarray:
    out, _ = run(inputs, npc=6250, npad=6272)
    return out



# revision 39
# speedup vs baseline: 1.8997x; 1.8997x over previous
"""Trainium2 Bass kernel for AttentiveGnLConv (TransformerConv + GRU + GATv2 + GRU).

Sharding: nodes partitioned across 8 cores (6250/core, padded to 6272 = 49*128).
Edges routed to the core owning the *target* (dst) node. Within each core,
nodes are greedy-packed into 49 blocks of <=128 so per-block edge counts are
balanced (minimizes chunk padding; K=6 chunks per block-half typical, the
few overflow blocks are ordered first so all 8 SPMD cores share one K array).

The SWDGE descriptor generation for indirect gathers (~8ns/descriptor on the
Pool engine) is the kernel bottleneck, so only the two irreducible per-edge
gathers remain (kv[src] in conv1, xl[src] in conv2); q[dst] and xr[dst] are
selected on-chip instead: transpose the per-chunk dst one-hot (PE), then
ohT.T @ q_block selects rows from SBUF-resident node-major tables.

Per-core phases:
  A. node tables: kv (k|v, all nodes, replicated compute, bf16, split lo/hi
     so A-half gathers can start early), q/skip/lin0 for own nodes (resident).
  B. conv1 edge pass: per 2-block group, gather kv[src]; e = ea @ te_w (PE);
     q[dst] via one-hot select; logits/softmax numerators on DVE; per-block
     scatter-add via one-hot matmul accumulated in PSUM; per-group batched
     normalize/transpose tail.
  C. node chain: elu -> GRU1 -> lin1; xl to DRAM, xr kept resident.
  D. AllGather xl shards -> full xl table.
  E. conv2 (GATv2) edge pass: gather xl[src], xr[dst] one-hot-selected and
     fused into zsum; same agg structure as B.
  F. node chain 2: elu -> GRU2 -> relu -> + g_sum -> output [64, 6272] f32
     (host unpermutes the block packing and concats).
"""

import sys

for _p in ("/opt/trn_rl_repo",):
    if _p not in sys.path:
        sys.path.insert(0, _p)

import math
from contextlib import ExitStack

import numpy as np
import ml_dtypes

import concourse.bass as bass
import concourse.bacc as bacc
import concourse.tile as tile
from concourse import mybir
from concourse.bass import IndirectOffsetOnAxis
from concourse.masks import make_identity
from concourse.library_config import mlp as mlp_lib

FP = mybir.dt.float32
BF = mybir.dt.bfloat16
F32R = mybir.dt.float32r
I32 = mybir.dt.int32
AF = mybir.ActivationFunctionType
OP = mybir.AluOpType

NCORES = 8
IN, HC, HID, H, C, EDIM = 64, 128, 64, 4, 32, 16
INV_SQRT_C = 1.0 / math.sqrt(C)

# bias pack column indices
B_TS, B_L0, B_G1R, B_G1Z, B_G1IN, B_G1HN, B_LIN1, B_GL, B_GR, B_GB, \
    B_G2R, B_G2Z, B_G2IN, B_G2HN, B_IOTA = range(15)
NB = 15


def _f32r(ap):
    return ap.bitcast(F32R)


def build_program(cfg):
    """Build the SPMD bass program. cfg: dict with
    npc (real nodes/core), npad (multiple of 128), K_blk (list, chunks per
    block), and chain chunk size ch (divides npad)."""
    npad = cfg["npad"]
    nblk = npad // 128
    nt = NCORES * npad
    half = nt // 2
    K_A = cfg["K_A"]
    K_B = cfg["K_B"]
    # groups of 2 blocks; slot order within group: A(b0) A(b1) B(b0) B(b1)
    ngrp = (nblk + 1) // 2
    grp_blocks = [tuple(b for b in (2 * g, 2 * g + 1) if b < nblk)
                  for g in range(ngrp)]
    # per-group chunk ranges
    grp_off = []     # start chunk (global slot-chunk index) of each group
    grp_KA = []      # total A chunks in group
    grp_K = []       # total chunks in group
    blk_chunks = {}  # block -> list of global chunk indices (A then B)
    pos = 0
    for g, bs in enumerate(grp_blocks):
        grp_off.append(pos)
        ka = sum(K_A[b] for b in bs)
        kb = sum(K_B[b] for b in bs)
        grp_KA.append(ka)
        grp_K.append(ka + kb)
        p = pos
        a_start = {}
        for b in bs:
            a_start[b] = p
            p += K_A[b]
        for b in bs:
            blk_chunks[b] = list(range(a_start[b], a_start[b] + K_A[b])) +                 list(range(p, p + K_B[b]))
            p += K_B[b]
        pos += ka + kb
    S = pos
    SA = sum(K_A)
    SB = sum(K_B)
    ch = cfg["ch"]
    nch = npad // ch
    assert nch * ch == npad

    nc = bacc.Bacc("TRN2", target_bir_lowering=False, debug=False,
                   num_devices=NCORES)

    # ---------------- DRAM parameters (inputs) ----------------
    def din(name, shape, dt):
        return nc.dram_tensor(name, shape, dt, kind="ExternalInput").ap()

    xt_full = din("xt_full", [IN, nt], BF)       # x^T, padded global layout
    xt_own = din("xt_own", [IN, npad], BF)       # per-core slice of xt_full
    idx_a = din("idx_a", [128, 8 * SA], mybir.dt.int16)   # src (A half)
    idx_b = din("idx_b", [128, 8 * SB], mybir.dt.int16)   # src-half (B half)
    dlocw = din("dlocw", [128, S], BF)           # dst-in-block (-1 pad)
    eat16 = din("eat16", [16, S * 128], BF)      # edge_attr^T per slot
    w_kv = din("w_kv", [IN, 2 * HC], BF)         # [tk_w | tv_w]
    w_q = din("w_q", [IN, HC], BF)
    w_te = din("w_te", [EDIM, HC], BF)
    w_ge = din("w_ge", [EDIM, HID], BF)
    w_sk = din("w_sk", [IN, HC], BF)             # ts_w
    w_l0 = din("w_l0", [IN, HC], BF)             # lin0_w
    w_g1 = din("w_g1", [HC, 6 * HC], BF)         # wi_r|wi_z|wi_n|wh_r|wh_z|wh_n
    w_lin1 = din("w_lin1", [HC, HID], BF)
    w_gl = din("w_gl", [HID, HID], BF)
    w_gr = din("w_gr", [HID, HID], BF)
    w_g2 = din("w_g2", [HID, 6 * HID], BF)
    biases = din("biases", [128, NB], FP)
    iota_in = din("iota_in", [128, 128], BF)     # row j value = j
    gatt_b = din("gatt_b", [128, HID], FP)       # g_att broadcast down parts
    kvb_b = din("kvb_b", [128, 4 * HC], FP)      # [tk_b|tv_b] broadcast
    tqb_b = din("tqb_b", [128, 2 * HC], FP)          # tq_b broadcast

    out_d = nc.dram_tensor("out", [HID, npad], FP, kind="ExternalOutput").ap()
    debug = cfg.get("debug", False)

    # ---------------- internal DRAM ----------------
    kv_table = nc.dram_tensor("kv_table", [nt, 2 * HC], BF).ap()
    xl_shard = nc.dram_tensor("xl_shard", [npad, HID], FP).ap()
    xl_table = nc.dram_tensor("xl_table", [nt, HID], FP,
                              addr_space="Shared").ap()

    if debug:
        dbg_kv = nc.dram_tensor("dbg_kv", [nt, 2 * HC], BF,
                                kind="ExternalOutput").ap()
        dbg_q = nc.dram_tensor("dbg_q", [128, nblk * HC], BF,
                               kind="ExternalOutput").ap()
        dbg_c1 = nc.dram_tensor("dbg_c1", [HC, npad], BF,
                                kind="ExternalOutput").ap()
        dbg_sk = nc.dram_tensor("dbg_sk", [HC, npad], BF,
                                kind="ExternalOutput").ap()
        dbg_x10 = nc.dram_tensor("dbg_x10", [HC, npad], BF,
                                 kind="ExternalOutput").ap()
        dbg_x1f = nc.dram_tensor("dbg_x1f", [HID, npad], BF,
                                 kind="ExternalOutput").ap()
        dbg_xlt = nc.dram_tensor("dbg_xlt", [nt, HID], FP,
                                 kind="ExternalOutput").ap()
        dbg_c2 = nc.dram_tensor("dbg_c2", [HID, npad], BF,
                                kind="ExternalOutput").ap()
        K0 = grp_K[0]
        dbg_kvg = nc.dram_tensor("dbg_kvg", [128, K0 * 256], BF,
                                 kind="ExternalOutput").ap()
        dbg_qg = nc.dram_tensor("dbg_qg", [128, K0 * 128], BF,
                                kind="ExternalOutput").ap()
        dbg_esb = nc.dram_tensor("dbg_esb", [128, K0 * 128], BF,
                                 kind="ExternalOutput").ap()
        dbg_oh = nc.dram_tensor("dbg_oh", [128, K0 * 128], BF,
                                kind="ExternalOutput").ap()
        dbg_msgp = nc.dram_tensor("dbg_msgp", [128, K0 * 132], BF,
                                  kind="ExternalOutput").ap()

    with tile.TileContext(nc) as tc, ExitStack() as top:
        const = top.enter_context(tc.tile_pool(name="const", bufs=1))

        # constants into SBUF
        idn_f = const.tile([128, 128], FP)
        make_identity(nc, idn_f[:])
        idn_b = const.tile([128, 128], BF)
        make_identity(nc, idn_b[:])
        c_wkv = const.tile([IN, 2 * HC], BF)
        nc.sync.dma_start(c_wkv[:], w_kv[:])
        c_wq = const.tile([IN, HC], BF)
        nc.sync.dma_start(c_wq[:], w_q[:])
        c_wte = const.tile([EDIM, HC], BF)
        nc.sync.dma_start(c_wte[:], w_te[:])
        c_wge = const.tile([EDIM, HID], BF)
        nc.sync.dma_start(c_wge[:], w_ge[:])
        c_wsk = const.tile([IN, HC], BF)
        nc.sync.dma_start(c_wsk[:], w_sk[:])
        c_wl0 = const.tile([IN, HC], BF)
        nc.sync.dma_start(c_wl0[:], w_l0[:])
        c_wg1 = const.tile([HC, 6 * HC], BF)
        nc.sync.dma_start(c_wg1[:], w_g1[:])
        c_wlin1 = const.tile([HC, HID], BF)
        nc.sync.dma_start(c_wlin1[:], w_lin1[:])
        c_wgl = const.tile([HID, HID], BF)
        nc.sync.dma_start(c_wgl[:], w_gl[:])
        c_wgr = const.tile([HID, HID], BF)
        nc.sync.dma_start(c_wgr[:], w_gr[:])
        c_wg2 = const.tile([HID, 6 * HID], BF)
        nc.sync.dma_start(c_wg2[:], w_g2[:])
        c_bias = const.tile([128, NB], FP)
        nc.sync.dma_start(c_bias[:], biases[:])
        c_iota = const.tile([128, 128], BF)
        nc.sync.dma_start(c_iota[:], iota_in[:])
        c_gatt = const.tile([128, HID], FP)
        nc.sync.dma_start(c_gatt[:], gatt_b[:])
        c_kvb = const.tile([128, 2 * HC], FP)
        nc.sync.dma_start(c_kvb[:], kvb_b[:])
        c_tqb = const.tile([128, 2 * HC], FP)
        nc.sync.dma_start(c_tqb[:], tqb_b[:])

        def bcol(j, p=128):
            return c_bias[:p, j:j + 1]

        # edge metadata resident
        meta = top.enter_context(tc.tile_pool(name="meta", bufs=1))
        m_ia = meta.tile([128, 8 * SA], mybir.dt.int16)
        nc.sync.dma_start(m_ia[:], idx_a[:])
        m_ib = meta.tile([128, 8 * SB], mybir.dt.int16)
        nc.sync.dma_start(m_ib[:], idx_b[:])
        m_dloc = meta.tile([128, S], BF)
        nc.sync.dma_start(m_dloc[:], dlocw[:])
        nc.gpsimd.load_library(mlp_lib)

        # resident per-node feature-major tensors
        res = top.enter_context(tc.tile_pool(name="res", bufs=1))
        r_x1fT = res.tile([HID, npad], BF)     # x1 final (= g_sum)
        r_qnm = res.tile([128, nblk * HC], BF)   # q node-major per block
        r_xrnm = res.tile([128, nblk * HID], BF)  # xr node-major per block

        resA_cm = tc.tile_pool(name="resA", bufs=1)
        resA = resA_cm.__enter__()
        r_skT = resA.tile([HC, npad], BF)      # x@ts_w (+ts_b later)
        r_x10T = resA.tile([HC, npad], BF)     # lrelu(x@lin0_w + b)
        r_c1aT = resA.tile([HC, npad], BF)     # conv1 aggregation^T (normed)

        # ---------------- stage A: tables ----------------
        with tc.tile_pool(name="stA", bufs=2) as pA, \
                tc.tile_pool(name="psA", bufs=2, space="PSUM") as psA:
            ntile_g = nt // 128
            assert ntile_g % 8 == 0
            for s8 in range(ntile_g // 8):      # 8 node-tiles per iteration
                xa = pA.tile([IN, 8 * 128], BF, tag="xa")
                nc.sync.dma_start(xa[:], xt_full[:, s8 * 1024:(s8 + 1) * 1024])
                stg = pA.tile([128, 8 * 256], BF, tag="kvstg")
                for j2 in range(4):             # psum pairs
                    ps = psA.tile([128, 512], FP, tag="pskv")
                    for j in range(2):
                        t = j2 * 2 + j
                        nc.tensor.matmul(ps[:, j * 256:(j + 1) * 256],
                                         lhsT=xa[:, t * 128:(t + 1) * 128],
                                         rhs=c_wkv[:], start=True, stop=True)
                    nc.vector.tensor_tensor(
                        out=stg[:, j2 * 512:(j2 + 1) * 512],
                        in0=ps[:], in1=c_kvb[:], op=OP.add)
                nc.sync.dma_start(
                    kv_table[s8 * 1024:(s8 + 1) * 1024, :].rearrange(
                        "(j p) f -> p j f", p=128),
                    stg[:].rearrange("p (j f) -> p j f", f=256))

            for t2 in range((nblk + 1) // 2):   # 2 blocks per iteration
                t = 2 * t2
                nb2 = min(2, nblk - t)
                w = nb2 * 128
                xo = pA.tile([IN, 256], BF, tag="xo")
                nc.sync.dma_start(xo[:, :w], xt_own[:, t * 128:t * 128 + w])
                psq = psA.tile([128, 256], FP, tag="psq")
                for j in range(nb2):
                    nc.tensor.matmul(psq[:, j * 128:(j + 1) * 128],
                                     lhsT=xo[:, j * 128:(j + 1) * 128],
                                     rhs=c_wq[:], start=True, stop=True)
                nc.vector.tensor_tensor(
                    out=r_qnm[:, t * HC:t * HC + w],
                    in0=psq[:, :w], in1=c_tqb[:, :w], op=OP.add)
                # feature-major skip / lin0 for own nodes
                pssl = psA.tile([128, 512], FP, tag="pssl")
                for j in range(nb2):
                    nc.tensor.matmul(pssl[:, j * 128:(j + 1) * 128],
                                     lhsT=c_wsk[:],
                                     rhs=xo[:, j * 128:(j + 1) * 128],
                                     start=True, stop=True)
                    nc.tensor.matmul(pssl[:, 256 + j * 128:256 + (j + 1) * 128],
                                     lhsT=c_wl0[:],
                                     rhs=xo[:, j * 128:(j + 1) * 128],
                                     start=True, stop=True)
                cols = slice(t * 128, t * 128 + w)
                nc.scalar.activation(r_skT[:, cols], pssl[:, 0:w], AF.Copy)
                tl0 = pA.tile([128, 256], FP, tag="tl0")
                nc.vector.tensor_scalar_add(tl0[:, :w], pssl[:, 256:256 + w],
                                            bcol(B_L0))
                nc.vector.scalar_tensor_tensor(
                    out=r_x10T[:, cols], in0=tl0[:, :w], scalar=0.01,
                    in1=tl0[:, :w], op0=OP.mult, op1=OP.max)

        # ---------------- stage B: conv1 edges + stage C chain ----------------
        with tc.tile_pool(name="stB", bufs=2) as pB, \
                tc.tile_pool(name="stBw", bufs=1) as pBw, \
                tc.tile_pool(name="psB", bufs=2, space="PSUM") as psB, \
                tc.tile_pool(name="psBa", bufs=1, space="PSUM") as psBa, \
                tc.tile_pool(name="psBq", bufs=1, space="PSUM") as psBq, \
                tc.tile_pool(name="stBs", bufs=1) as pBs:
            for g in range(ngrp):
                Kg = grp_K[g]
                KAg = grp_KA[g]
                KBg = Kg - KAg
                c0 = grp_off[g]
                kvg = pB.tile([128, Kg * 256], BF, tag="kvg")
                kv3 = kvg[:].rearrange("p (k f) -> p k f", f=256)
                nc.gpsimd.dma_gather(
                    kv3[:, 0:KAg, :], kv_table[0:half, :],
                    m_ia[:, 8 * sum(grp_KA[:g]):8 * (sum(grp_KA[:g]) + KAg)],
                    KAg * 128, KAg * 128, 256, single_packet=False)
                b0s = 8 * (sum(grp_K[:g]) - sum(grp_KA[:g]))
                nc.gpsimd.dma_gather(
                    kv3[:, KAg:Kg, :], kv_table[half:nt, :],
                    m_ib[:, b0s:b0s + 8 * KBg],
                    KBg * 128, KBg * 128, 256, single_packet=False)
                # ---- on-chip q[dst] selection (replaces DMA gather) ----
                blk_of = {}
                for b in grp_blocks[g]:
                    for cc in blk_chunks[b]:
                        blk_of[cc - c0] = b
                oh = pBw.tile([128, Kg * 128], BF, tag="oh")
                nc.vector.tensor_tensor(
                    out=oh[:].rearrange("p (k f) -> p k f", f=128),
                    in0=c_iota[:].unsqueeze(1).to_broadcast([128, Kg, 128]),
                    in1=m_dloc[:, c0:c0 + Kg].unsqueeze(2).to_broadcast(
                        [128, Kg, 128]),
                    op=OP.is_equal)
                ohT = pBw.tile([128, Kg * 128], BF, tag="ohT")
                qg = pBw.tile([128, Kg * 128], BF, tag="qg")
                for q4 in range((Kg + 3) // 4):
                    kk = min(4, Kg - q4 * 4)
                    psb = psBq.tile([128, 512], BF, tag="psg4")
                    for j in range(kk):
                        k = q4 * 4 + j
                        nc.tensor.transpose(psb[:, j * 128:(j + 1) * 128],
                                            oh[:, k * 128:(k + 1) * 128],
                                            idn_b[:])
                    nc.vector.tensor_scalar(
                        out=ohT[:, q4 * 512:q4 * 512 + kk * 128],
                        in0=psb[:, :kk * 128], scalar1=1.0,
                        scalar2=None, op0=OP.mult)
                for q4 in range((Kg + 3) // 4):
                    kk = min(4, Kg - q4 * 4)
                    psq4 = psBq.tile([128, 512], FP, tag="psq4")
                    for j in range(kk):
                        k = q4 * 4 + j
                        nc.tensor.matmul(
                            psq4[:, j * 128:(j + 1) * 128],
                            lhsT=ohT[:, k * 128:(k + 1) * 128],
                            rhs=r_qnm[:, blk_of[k] * HC:(blk_of[k] + 1) * HC],
                            start=True, stop=True)
                    nc.vector.tensor_scalar(
                        out=qg[:, q4 * 512:q4 * 512 + kk * 128],
                        in0=psq4[:, :kk * 128], scalar1=1.0,
                        scalar2=None, op0=OP.mult)
                eat = pB.tile([16, Kg * 128], BF, tag="eat")
                nc.scalar.dma_start(eat[:],
                                    eat16[:, c0 * 128:(c0 + Kg) * 128])
                e_sb = pBw.tile([128, Kg * 128], BF, tag="esb")
                for q4 in range((Kg + 3) // 4):
                    kk = min(4, Kg - q4 * 4)
                    pse = psB.tile([128, 512], FP, tag="pse")
                    for cc in range(kk):
                        ca = q4 * 4 + cc
                        nc.tensor.matmul(
                            pse[:, cc * 128:(cc + 1) * 128],
                            lhsT=eat[:, ca * 128:(ca + 1) * 128],
                            rhs=c_wte[:], start=True, stop=True)
                    nc.scalar.activation(
                        e_sb[:, q4 * 512:q4 * 512 + kk * 128],
                        pse[:, :kk * 128], AF.Copy)
                keye = pBw.tile([128, Kg * 128], BF, tag="keye")
                k3 = keye[:].rearrange("p (k f) -> p k f", f=128)
                e3 = e_sb[:].rearrange("p (k f) -> p k f", f=128)
                nc.vector.tensor_tensor(out=k3, in0=kv3[:, :, 0:128], in1=e3,
                                        op=OP.add)
                prod = pBw.tile([128, Kg * 128], BF, tag="prod")
                nc.vector.tensor_tensor(
                    out=prod[:], in0=keye[:], in1=qg[:], op=OP.mult)
                lgt = pBw.tile([128, Kg * 4], FP, tag="lgt")
                nc.vector.tensor_reduce(
                    lgt[:].rearrange("p (k h) -> p k h", h=4),
                    prod[:].rearrange("p (k h c) -> p k h c", h=4, c=32),
                    axis=mybir.AxisListType.X, op=OP.add)
                msgp = pBw.tile([128, Kg * 132], BF, tag="msgp")
                mp3 = msgp[:].rearrange("p (k f) -> p k f", f=132)
                nc.scalar.activation(
                    mp3[:, :, 128:132],
                    lgt[:].rearrange("p (k h) -> p k h", h=4),
                    AF.Exp, scale=INV_SQRT_C)
                msgv = pBw.tile([128, Kg * 128], BF, tag="keye")
                mv3 = msgv[:].rearrange("p (k f) -> p k f", f=128)
                nc.vector.tensor_tensor(out=mv3, in0=kv3[:, :, 128:256],
                                        in1=e3, op=OP.add)
                nc.vector.tensor_tensor(
                    out=mp3[:, :, 0:128].rearrange("p k (h c) -> p k h c",
                                                   c=32),
                    in0=msgv[:].rearrange("p (k h c) -> p k h c", h=4, c=32),
                    in1=mp3[:, :, 128:132].unsqueeze(3).to_broadcast(
                        [128, Kg, 4, 32]),
                    op=OP.mult)
                if debug and g == 0:
                    nc.sync.dma_start(dbg_kvg[:], kvg[:, :dbg_kvg.shape[1]])
                    nc.sync.dma_start(dbg_qg[:], qg[:, :dbg_qg.shape[1]])
                    nc.sync.dma_start(dbg_esb[:], e_sb[:, :dbg_esb.shape[1]])
                    nc.sync.dma_start(dbg_oh[:], oh[:, :dbg_oh.shape[1]])
                    nc.sync.dma_start(dbg_msgp[:],
                                      msgp[:, :dbg_msgp.shape[1]])
                nbg = len(grp_blocks[g])
                pagg = psBa.tile([128, nbg * 132], FP, tag="agg")
                for bi, b in enumerate(grp_blocks[g]):
                    chs = [cc - c0 for cc in blk_chunks[b]]
                    for ci, cc in enumerate(chs):
                        nc.tensor.matmul(
                            pagg[:, bi * 132:(bi + 1) * 132],
                            lhsT=oh[:, cc * 128:(cc + 1) * 128],
                            rhs=msgp[:, cc * 132:(cc + 1) * 132],
                            start=(ci == 0), stop=(ci == len(chs) - 1))
                p3 = pagg[:].rearrange("p (b f) -> p b f", f=132)
                den = pBs.tile([128, 2 * H], FP, tag="den")
                nc.vector.tensor_scalar_add(
                    den[:, :nbg * H].rearrange("p (b h) -> p b h", h=H),
                    p3[:, :, 128:132], 1e-16)
                rec = pBs.tile([128, 2 * H], FP, tag="rec")
                nc.vector.reciprocal(rec[:, :nbg * H], den[:, :nbg * H])
                aggn = pBs.tile([128, 256], BF, tag="aggn")
                nc.vector.tensor_tensor(
                    out=aggn[:, :nbg * 128].rearrange(
                        "p (b h c) -> p b h c", h=H, c=C),
                    in0=p3[:, :, 0:128].rearrange(
                        "p b (h c) -> p b h c", c=C),
                    in1=rec[:, :nbg * H].rearrange(
                        "p (b h) -> p b h", h=H).unsqueeze(3).to_broadcast(
                        [128, nbg, H, C]),
                    op=OP.mult)
                pst = psBa.tile([128, 256], BF, tag="ptr1")
                for bi in range(nbg):
                    nc.tensor.transpose(pst[:, bi * 128:(bi + 1) * 128],
                                        aggn[:, bi * 128:(bi + 1) * 128],
                                        idn_b[:])
                b0 = grp_blocks[g][0]
                nc.scalar.activation(
                    r_c1aT[:, b0 * 128:b0 * 128 + nbg * 128],
                    pst[:, :nbg * 128], AF.Copy)

            # ---- stage C: chain 1 (feature-major, chunks of ch) ----
            for t in range(nch):
                cols = slice(t * ch, (t + 1) * ch)
                c1 = pBs.tile([128, ch], FP, tag="c1")
                nc.vector.tensor_tensor(out=c1[:], in0=r_c1aT[:, cols],
                                        in1=r_skT[:, cols], op=OP.add)
                mn = pBs.tile([128, ch], FP, tag="mn")
                nc.vector.tensor_scalar(out=mn[:], in0=c1[:],
                                        scalar1=bcol(B_TS), scalar2=0.0,
                                        op0=OP.add, op1=OP.min)
                mx = pBs.tile([128, ch], FP, tag="mx")
                nc.vector.tensor_scalar(out=mx[:], in0=c1[:],
                                        scalar1=bcol(B_TS), scalar2=0.0,
                                        op0=OP.add, op1=OP.max)
                em = pBs.tile([128, ch], FP, tag="em")
                nc.scalar.activation(em[:], mn[:], AF.Exp)
                hh = pBs.tile([128, ch], BF, tag="hh")
                nc.vector.scalar_tensor_tensor(
                    out=hh[:], in0=em[:], scalar=-1.0, in1=mx[:],
                    op0=OP.add, op1=OP.add)
                pr = psB.tile([128, ch], FP, tag="pg")
                nc.tensor.matmul(pr[:], lhsT=c_wg1[:, 0:128],
                                 rhs=hh[:], start=True, stop=False)
                nc.tensor.matmul(pr[:], lhsT=c_wg1[:, 384:512],
                                 rhs=r_x10T[:, cols],
                                 start=False, stop=True)
                pz = psB.tile([128, ch], FP, tag="pg")
                nc.tensor.matmul(pz[:], lhsT=c_wg1[:, 128:256],
                                 rhs=hh[:], start=True, stop=False)
                nc.tensor.matmul(pz[:], lhsT=c_wg1[:, 512:640],
                                 rhs=r_x10T[:, cols],
                                 start=False, stop=True)
                pin = psB.tile([128, ch], FP, tag="pg")
                nc.tensor.matmul(pin[:], lhsT=c_wg1[:, 256:384],
                                 rhs=hh[:], start=True, stop=True)
                phn = psB.tile([128, ch], FP, tag="pg")
                nc.tensor.matmul(phn[:], lhsT=c_wg1[:, 640:768],
                                 rhs=r_x10T[:, cols],
                                 start=True, stop=True)
                r_ = pBs.tile([128, ch], FP, tag="r_")
                nc.scalar.activation(r_[:], pr[:], AF.Sigmoid,
                                     bias=bcol(B_G1R))
                z_ = pBs.tile([128, ch], FP, tag="z_")
                nc.scalar.activation(z_[:], pz[:], AF.Sigmoid,
                                     bias=bcol(B_G1Z))
                thn = pBs.tile([128, ch], FP, tag="thn")
                nc.vector.scalar_tensor_tensor(
                    out=thn[:], in0=phn[:], scalar=bcol(B_G1HN), in1=r_[:],
                    op0=OP.add, op1=OP.mult)
                pre = pBs.tile([128, ch], FP, tag="pre")
                nc.vector.tensor_tensor(out=pre[:], in0=pin[:], in1=thn[:],
                                        op=OP.add)
                n_ = pBs.tile([128, ch], FP, tag="n_")
                nc.scalar.activation(n_[:], pre[:], AF.Tanh,
                                     bias=bcol(B_G1IN))
                d_ = pBs.tile([128, ch], FP, tag="d_")
                nc.vector.tensor_tensor(out=d_[:], in0=r_x10T[:, cols],
                                        in1=n_[:], op=OP.subtract)
                zd = pBs.tile([128, ch], FP, tag="zd")
                nc.vector.tensor_tensor(out=zd[:], in0=z_[:], in1=d_[:],
                                        op=OP.mult)
                x1g = pBs.tile([128, ch], FP, tag="x1g")
                nc.vector.tensor_tensor(out=x1g[:], in0=n_[:], in1=zd[:],
                                        op=OP.add)
                x1r = pBs.tile([128, ch], BF, tag="x1r")
                nc.scalar.activation(x1r[:], x1g[:], AF.Relu)
                psl = psBa.tile([HID, ch], FP, tag="psx")
                nc.tensor.matmul(psl[:], lhsT=c_wlin1[:],
                                 rhs=x1r[:], start=True, stop=True)
                tl1 = pBs.tile([HID, ch], FP, tag="tl1")
                nc.vector.tensor_scalar_add(tl1[:], psl[:],
                                            bcol(B_LIN1, HID))
                nc.vector.scalar_tensor_tensor(
                    out=r_x1fT[:, cols], in0=tl1[:], scalar=0.01, in1=tl1[:],
                    op0=OP.mult, op1=OP.max)

            # ---- xl / xr shards (node-major), per 2 blocks ----
            for b2 in range(nblk // 2 + (nblk % 2)):
                b = b2 * 2
                nb2 = min(2, nblk - b)
                w = nb2 * 128
                cols = slice(b * 128, b * 128 + w)
                psx = psBa.tile([HID, 512], FP, tag="psx")
                nc.tensor.matmul(psx[:, 0:w], lhsT=c_wgl[:],
                                 rhs=r_x1fT[:, cols],
                                 start=True, stop=True)
                nc.tensor.matmul(psx[:, 256:256 + w], lhsT=c_wgr[:],
                                 rhs=r_x1fT[:, cols],
                                 start=True, stop=True)
                xls = pBs.tile([HID, 256], FP, tag="xls")
                nc.vector.tensor_scalar_add(xls[:, :w], psx[:, 0:w],
                                            bcol(B_GL, HID))
                xrs = pBs.tile([HID, 256], FP, tag="xrs")
                nc.vector.tensor_scalar_add(xrs[:, :w], psx[:, 256:256 + w],
                                            bcol(B_GR, HID))
                ptx = psBa.tile([128, 256], FP, tag="ptr2")
                for j in range(nb2):
                    nc.tensor.transpose(ptx[:, j * 64:(j + 1) * 64],
                                        xls[:, j * 128:(j + 1) * 128],
                                        idn_f[:HID, :HID])
                    nc.tensor.transpose(ptx[:, 128 + j * 64:128 + (j + 1) * 64],
                                        xrs[:, j * 128:(j + 1) * 128],
                                        idn_f[:HID, :HID])
                stx = pBs.tile([128, 128], FP, tag="stx")
                nc.scalar.activation(stx[:, 0:nb2 * 64],
                                     ptx[:, 0:nb2 * 64], AF.Copy)
                nc.scalar.activation(r_xrnm[:, b * HID:(b + nb2) * HID],
                                     ptx[:, 128:128 + nb2 * 64], AF.Copy)
                nc.scalar.dma_start(
                    xl_shard[b * 128:b * 128 + w, :].rearrange(
                        "(j p) f -> p j f", p=128),
                    stx[:].rearrange("p (j f) -> p j f", f=64)[:, 0:nb2])

        if debug:
            nc.sync.dma_start(dbg_c1[:], r_c1aT[:])
            nc.sync.dma_start(dbg_sk[:], r_skT[:])
            nc.sync.dma_start(dbg_x10[:], r_x10T[:])
        resA_cm.__exit__(None, None, None)

        # ---------------- stage D: AllGather xl ----------------
        nc.gpsimd.collective_compute(
            "AllGather", OP.bypass,
            ins=[xl_shard[:]], outs=[xl_table[:]],
            replica_groups=[list(range(NCORES))])

        # ---------------- stage E: conv2 edges + chain 2 ----------------
        with tc.tile_pool(name="stE", bufs=2) as pE, \
                tc.tile_pool(name="stEw", bufs=1) as pEw, \
                tc.tile_pool(name="psE", bufs=2, space="PSUM") as psE, \
                tc.tile_pool(name="psEa", bufs=1, space="PSUM") as psEa, \
                tc.tile_pool(name="psEq", bufs=1, space="PSUM") as psEq, \
                tc.tile_pool(name="stEs", bufs=1) as pEs:
            r_c2aT = pEs.tile([HID, npad], BF, tag="c2aT")
            for g in range(ngrp):
                Kg = grp_K[g]
                KAg = grp_KA[g]
                KBg = Kg - KAg
                c0 = grp_off[g]
                xlg = pE.tile([128, Kg * 64], FP, tag="xlg")
                xl3 = xlg[:].rearrange("p (k f) -> p k f", f=64)
                nc.gpsimd.dma_gather(
                    xl3[:, 0:KAg, :], xl_table[0:half, :],
                    m_ia[:, 8 * sum(grp_KA[:g]):8 * (sum(grp_KA[:g]) + KAg)],
                    KAg * 128, KAg * 128, 64, single_packet=False)
                b0s = 8 * (sum(grp_K[:g]) - sum(grp_KA[:g]))
                nc.gpsimd.dma_gather(
                    xl3[:, KAg:Kg, :], xl_table[half:nt, :],
                    m_ib[:, b0s:b0s + 8 * KBg],
                    KBg * 128, KBg * 128, 64, single_packet=False)
                # ---- on-chip xr[dst] selection (replaces DMA gather) ----
                blk_of = {}
                for b in grp_blocks[g]:
                    for cc in blk_chunks[b]:
                        blk_of[cc - c0] = b
                oh2 = pEw.tile([128, Kg * 128], BF, tag="oh2")
                nc.vector.tensor_tensor(
                    out=oh2[:].rearrange("p (k f) -> p k f", f=128),
                    in0=c_iota[:].unsqueeze(1).to_broadcast([128, Kg, 128]),
                    in1=m_dloc[:, c0:c0 + Kg].unsqueeze(2).to_broadcast(
                        [128, Kg, 128]),
                    op=OP.is_equal)
                ohT2 = pEw.tile([128, Kg * 128], BF, tag="ohT2")
                zsum = pEw.tile([128, Kg * 64], FP, tag="zsum")
                for q4 in range((Kg + 3) // 4):
                    kk = min(4, Kg - q4 * 4)
                    psb = psEq.tile([128, 512], BF, tag="psg4")
                    for j in range(kk):
                        k = q4 * 4 + j
                        nc.tensor.transpose(psb[:, j * 128:(j + 1) * 128],
                                            oh2[:, k * 128:(k + 1) * 128],
                                            idn_b[:])
                    nc.vector.tensor_scalar(
                        out=ohT2[:, q4 * 512:q4 * 512 + kk * 128],
                        in0=psb[:, :kk * 128], scalar1=1.0,
                        scalar2=None, op0=OP.mult)
                for q8 in range((Kg + 7) // 8):
                    kk = min(8, Kg - q8 * 8)
                    psx2 = psEq.tile([128, 512], FP, tag="psq4")
                    for j in range(kk):
                        k = q8 * 8 + j
                        nc.tensor.matmul(
                            psx2[:, j * 64:(j + 1) * 64],
                            lhsT=ohT2[:, k * 128:(k + 1) * 128],
                            rhs=r_xrnm[:, blk_of[k] * HID:
                                       (blk_of[k] + 1) * HID],
                            start=True, stop=True)
                    nc.vector.tensor_tensor(
                        out=zsum[:, q8 * 512:q8 * 512 + kk * 64],
                        in0=xlg[:, q8 * 512:q8 * 512 + kk * 64],
                        in1=psx2[:, :kk * 64], op=OP.add)
                eat = pE.tile([16, Kg * 128], BF, tag="eat")
                nc.scalar.dma_start(eat[:],
                                    eat16[:, c0 * 128:(c0 + Kg) * 128])
                zed = pEw.tile([128, Kg * 64], FP, tag="zed")
                for q4 in range((Kg + 3) // 4):
                    kk = min(4, Kg - q4 * 4)
                    pse2 = psE.tile([128, 256], FP, tag="pse2")
                    for cc in range(kk):
                        ca = q4 * 4 + cc
                        nc.tensor.matmul(
                            pse2[:, cc * 64:(cc + 1) * 64],
                            lhsT=eat[:, ca * 128:(ca + 1) * 128],
                            rhs=c_wge[:], start=True, stop=True)
                    nc.vector.tensor_tensor(
                        out=zed[:, q4 * 256:q4 * 256 + kk * 64],
                        in0=pse2[:, :kk * 64],
                        in1=zsum[:, q4 * 256:q4 * 256 + kk * 64], op=OP.add)
                zl = pEw.tile([128, Kg * 64], FP, tag="zl")
                nc.vector.scalar_tensor_tensor(
                    out=zl[:], in0=zed[:], scalar=0.2, in1=zed[:],
                    op0=OP.mult, op1=OP.max)
                pr2 = pEw.tile([128, Kg * 64], FP, tag="pr2")
                nc.vector.tensor_tensor(
                    out=pr2[:].rearrange("p (k f) -> p k f", f=64),
                    in0=zl[:].rearrange("p (k f) -> p k f", f=64),
                    in1=c_gatt[:, :64].unsqueeze(1).to_broadcast(
                        [128, Kg, 64]),
                    op=OP.mult)
                lg2 = pEw.tile([128, Kg], FP, tag="lg2")
                nc.vector.tensor_reduce(
                    lg2[:].unsqueeze(2),
                    pr2[:].rearrange("p (k f) -> p k f", f=64),
                    axis=mybir.AxisListType.X, op=OP.add)
                mq2 = pEw.tile([128, Kg * 65], BF, tag="mq2")
                m23 = mq2[:].rearrange("p (k f) -> p k f", f=65)
                nc.scalar.activation(m23[:, :, 64:65],
                                     lg2[:].unsqueeze(2), AF.Exp)
                nc.vector.tensor_tensor(
                    out=m23[:, :, 0:64],
                    in0=xlg[:].rearrange("p (k f) -> p k f", f=64),
                    in1=m23[:, :, 64:65].to_broadcast([128, Kg, 64]),
                    op=OP.mult)
                nbg = len(grp_blocks[g])
                pagg2 = psEa.tile([128, nbg * 65], FP, tag="agg2")
                for bi, b in enumerate(grp_blocks[g]):
                    chs = [cc - c0 for cc in blk_chunks[b]]
                    for ci, cc in enumerate(chs):
                        nc.tensor.matmul(
                            pagg2[:, bi * 65:(bi + 1) * 65],
                            lhsT=oh2[:, cc * 128:(cc + 1) * 128],
                            rhs=mq2[:, cc * 65:(cc + 1) * 65],
                            start=(ci == 0), stop=(ci == len(chs) - 1))
                p32 = pagg2[:].rearrange("p (b f) -> p b f", f=65)
                den2 = pEs.tile([128, 2], FP, tag="den2")
                nc.vector.tensor_scalar_add(
                    den2[:, :nbg].unsqueeze(2), p32[:, :, 64:65], 1e-16)
                rec2 = pEs.tile([128, 2], FP, tag="rec2")
                nc.vector.reciprocal(rec2[:, :nbg], den2[:, :nbg])
                ag2 = pEs.tile([128, 128], BF, tag="ag2")
                nc.vector.tensor_tensor(
                    out=ag2[:, :nbg * 64].rearrange(
                        "p (b f) -> p b f", f=64),
                    in0=p32[:, :, 0:64],
                    in1=rec2[:, :nbg].unsqueeze(2).to_broadcast(
                        [128, nbg, 64]),
                    op=OP.mult)
                pst2 = psEa.tile([HID, 256], BF, tag="ptr3")
                for bi in range(nbg):
                    nc.tensor.transpose(pst2[:, bi * 128:(bi + 1) * 128],
                                        ag2[:, bi * 64:(bi + 1) * 64],
                                        idn_b[:])
                b0 = grp_blocks[g][0]
                nc.scalar.activation(
                    r_c2aT[:, b0 * 128:b0 * 128 + nbg * 128],
                    pst2[:, :nbg * 128], AF.Copy)
            if debug:
                nc.sync.dma_start(dbg_c2[:], r_c2aT[:])

            # ---- chain 2 ----
            for t in range(nch):
                cols = slice(t * ch, (t + 1) * ch)
                mn2 = pEs.tile([HID, ch], FP, tag="mn2")
                nc.vector.tensor_scalar(out=mn2[:], in0=r_c2aT[:HID, cols],
                                        scalar1=bcol(B_GB, HID), scalar2=0.0,
                                        op0=OP.add, op1=OP.min)
                mx2 = pEs.tile([HID, ch], FP, tag="mx2")
                nc.vector.tensor_scalar(out=mx2[:], in0=r_c2aT[:HID, cols],
                                        scalar1=bcol(B_GB, HID), scalar2=0.0,
                                        op0=OP.add, op1=OP.max)
                em2 = pEs.tile([HID, ch], FP, tag="em2")
                nc.scalar.activation(em2[:], mn2[:], AF.Exp)
                h2 = pEs.tile([HID, ch], BF, tag="h2")
                nc.vector.scalar_tensor_tensor(
                    out=h2[:], in0=em2[:], scalar=-1.0, in1=mx2[:],
                    op0=OP.add, op1=OP.add)
                pr = psE.tile([HID, ch], FP, tag="pg2")
                nc.tensor.matmul(pr[:], lhsT=c_wg2[:, 0:64],
                                 rhs=h2[:], start=True, stop=False)
                nc.tensor.matmul(pr[:], lhsT=c_wg2[:, 192:256],
                                 rhs=r_x1fT[:, cols],
                                 start=False, stop=True)
                pz = psE.tile([HID, ch], FP, tag="pg2")
                nc.tensor.matmul(pz[:], lhsT=c_wg2[:, 64:128],
                                 rhs=h2[:], start=True, stop=False)
                nc.tensor.matmul(pz[:], lhsT=c_wg2[:, 256:320],
                                 rhs=r_x1fT[:, cols],
                                 start=False, stop=True)
                pin = psE.tile([HID, ch], FP, tag="pg2")
                nc.tensor.matmul(pin[:], lhsT=c_wg2[:, 128:192],
                                 rhs=h2[:], start=True, stop=True)
                phn = psE.tile([HID, ch], FP, tag="pg2")
                nc.tensor.matmul(phn[:], lhsT=c_wg2[:, 320:384],
                                 rhs=r_x1fT[:, cols],
                                 start=True, stop=True)
                r2 = pEs.tile([HID, ch], FP, tag="r2")
                nc.scalar.activation(r2[:], pr[:], AF.Sigmoid,
                                     bias=bcol(B_G2R, HID))
                z2 = pEs.tile([HID, ch], FP, tag="z2")
                nc.scalar.activation(z2[:], pz[:], AF.Sigmoid,
                                     bias=bcol(B_G2Z, HID))
                th2 = pEs.tile([HID, ch], FP, tag="th2")
                nc.vector.scalar_tensor_tensor(
                    out=th2[:], in0=phn[:], scalar=bcol(B_G2HN, HID),
                    in1=r2[:], op0=OP.add, op1=OP.mult)
                pre2 = pEs.tile([HID, ch], FP, tag="pre2")
                nc.vector.tensor_tensor(out=pre2[:], in0=pin[:], in1=th2[:],
                                        op=OP.add)
                n2 = pEs.tile([HID, ch], FP, tag="n2")
                nc.scalar.activation(n2[:], pre2[:], AF.Tanh,
                                     bias=bcol(B_G2IN, HID))
                d2 = pEs.tile([HID, ch], FP, tag="d2")
                nc.vector.tensor_tensor(out=d2[:], in0=r_x1fT[:, cols],
                                        in1=n2[:], op=OP.subtract)
                zd2 = pEs.tile([HID, ch], FP, tag="zd2")
                nc.vector.tensor_tensor(out=zd2[:], in0=z2[:], in1=d2[:],
                                        op=OP.mult)
                x2g = pEs.tile([HID, ch], FP, tag="x2g")
                nc.vector.tensor_tensor(out=x2g[:], in0=n2[:], in1=zd2[:],
                                        op=OP.add)
                x2r = pEs.tile([HID, ch], FP, tag="x2r")
                nc.scalar.activation(x2r[:], x2g[:], AF.Relu)
                oo = pEs.tile([HID, ch], FP, tag="oo")
                nc.vector.tensor_tensor(out=oo[:], in0=x2r[:],
                                        in1=r_x1fT[:, cols], op=OP.add)
                nc.sync.dma_start(out_d[:, cols], oo[:])

        if debug:
            nc.sync.dma_start(dbg_kv[:], kv_table[:])
            nc.sync.dma_start(dbg_q[:], r_qnm[:])
            nc.sync.dma_start(dbg_x1f[:], r_x1fT[:])
            nc.sync.dma_start(dbg_xlt[:], xl_table[:])

    nc.compile()
    return nc


# ======================= host-side data prep =======================

def _pack_blocks(degA, degB, nblk):
    """Greedy-balance nodes into blocks of <=128 so that per-block A/B edge
    loads are near-equal (minimizes chunk padding). Returns per-node packed
    position (block*128 + rank)."""
    npc = len(degA)
    loadsA = np.zeros(nblk)
    loadsB = np.zeros(nblk)
    counts = np.zeros(nblk, np.int64)
    assign = np.zeros(npc, np.int64)
    order = np.argsort(-(degA + degB), kind="stable")
    # overflow blocks (cap 896 = 7 chunks) absorb the mass beyond 49*768
    exA = max(0.0, degA.sum() - nblk * 768.0)
    exB = max(0.0, degB.sum() - nblk * 768.0)
    nover = int(max(np.ceil(exA / 128.0), np.ceil(exB / 128.0)))
    capA = np.full(nblk, 768.0)
    capB = np.full(nblk, 768.0)
    capA[:nover] = 896.0
    capB[:nover] = 896.0
    for n_ in order:
        da, db = degA[n_], degB[n_]
        over = (np.maximum(loadsA + da - capA, 0.0)
                + np.maximum(loadsB + db - capB, 0.0)) * 1e6
        score = over + np.maximum(loadsA + da - capA, loadsB + db - capB) \
            + 0.001 * (loadsA + loadsB)
        score[counts >= 128] = 1e18
        b = int(np.argmin(score))
        assign[n_] = b
        loadsA[b] += da
        loadsB[b] += db
        counts[b] += 1
    # repair: move light nodes out of over-cap blocks when possible
    for _ in range(4):
        moved = False
        for loads, caps in ((loadsA, capA), (loadsB, capB)):
            for b in np.flatnonzero(loads > caps):
                members = np.flatnonzero(assign == b)
                members = members[np.argsort(degA[members] + degB[members])]
                for n_ in members:
                    if loads[b] <= caps[b]:
                        break
                    da, db = degA[n_], degB[n_]
                    ok = ((counts < 128) & (loadsA + da <= capA)
                          & (loadsB + db <= capB))
                    ok[b] = False
                    cand = np.flatnonzero(ok)
                    if len(cand):
                        t = int(cand[np.argmin((loadsA + loadsB)[cand]
                                               - (capA + capB)[cand])])
                        assign[n_] = t
                        loadsA[b] -= da
                        loadsB[b] -= db
                        counts[b] -= 1
                        loadsA[t] += da
                        loadsB[t] += db
                        counts[t] += 1
                        moved = True
        if not moved:
            break
    # order blocks heavy-first so padded (K=7) blocks align across cores
    key = np.ceil(loadsA / 128) * 1000 + np.ceil(loadsB / 128)
    blk_order = np.argsort(-(key + (loadsA + loadsB) * 1e-6), kind="stable")
    rank_of = np.empty(nblk, np.int64)
    rank_of[blk_order] = np.arange(nblk)
    pos = np.zeros(npc, np.int64)
    nxt = np.zeros(nblk, np.int64)
    for n_ in range(npc):
        b = assign[n_]
        pos[n_] = rank_of[b] * 128 + nxt[b]
        nxt[b] += 1
    return pos


def prep_inputs(inputs, npc, npad):
    """Build per-core in_maps + cfg from the full problem inputs."""
    x = np.asarray(inputs["x"], np.float32)
    ea = np.asarray(inputs["edge_attr"], np.float32)
    ei = np.asarray(inputs["edge_index"], np.int64)
    n = x.shape[0]
    assert n == NCORES * npc
    nblk = npad // 128
    nt = NCORES * npad
    half = nt // 2
    src, dst = ei[0], ei[1]
    dcore = dst // npc
    scoreg = src // npc
    # balanced node->slot packing per core (reduces chunk padding)
    isA_by_src = ((src // npc) * npad + (src % npc)) < half
    perm = np.zeros((NCORES, npc), np.int64)
    for c in range(NCORES):
        m = dcore == c
        dl = dst[m] - c * npc
        degA = np.bincount(dl[isA_by_src[m]], minlength=npc).astype(np.float64)
        degB = np.bincount(dl[~isA_by_src[m]],
                           minlength=npc).astype(np.float64)
        perm[c] = _pack_blocks(degA, degB, nblk)
    dloc_in_core = perm[dcore, dst - dcore * npc]
    blk = dloc_in_core // 128
    dloc = dloc_in_core % 128
    srcp = scoreg * npad + perm[scoreg, src - scoreg * npc]  # packed global id
    isA = srcp < half

    order = np.lexsort((blk, dcore))
    keys = dcore[order] * nblk + blk[order]
    bounds = np.searchsorted(keys, np.arange(NCORES * nblk + 1))
    el_A, el_B = [], []
    KA_cb = np.zeros((NCORES, nblk), np.int64)
    KB_cb = np.zeros((NCORES, nblk), np.int64)
    for c in range(NCORES):
        for b in range(nblk):
            j = c * nblk + b
            el = order[bounds[j]:bounds[j + 1]]
            a = el[isA[el]]
            bb = el[~isA[el]]
            el_A.append(a)
            el_B.append(bb)
            KA_cb[c, b] = (len(a) + 127) // 128
            KB_cb[c, b] = (len(bb) + 127) // 128
    K_A = [int(max(1, KA_cb[:, b].max())) for b in range(nblk)]
    K_B = [int(max(1, KB_cb[:, b].max())) for b in range(nblk)]
    ngrp = (nblk + 1) // 2
    grp_blocks = [tuple(b for b in (2 * g, 2 * g + 1) if b < nblk)
                  for g in range(ngrp)]
    S = sum(K_A) + sum(K_B)
    SA = sum(K_A)
    SB = sum(K_B)

    def tile16(a):
        w = a.reshape(-1, 16).T
        return np.ascontiguousarray(np.tile(w, (8, 1)))

    per_core = []
    for c in range(NCORES):
        dlocs = np.full(S * 128, -1.0, np.float32)
        eas = np.zeros((S * 128, EDIM), np.float32)
        ia = np.zeros(SA * 128, np.int64)
        ib = np.zeros(SB * 128, np.int64)
        pos = apos = bpos = 0
        for g in range(ngrp):
            for b in grp_blocks[g]:           # A slots
                el = el_A[c * nblk + b]
                ne = len(el)
                dlocs[pos:pos + ne] = dloc[el]
                eas[pos:pos + ne] = ea[el]
                ia[apos:apos + ne] = srcp[el]
                pos += K_A[b] * 128
                apos += K_A[b] * 128
            for b in grp_blocks[g]:           # B slots
                el = el_B[c * nblk + b]
                ne = len(el)
                dlocs[pos:pos + ne] = dloc[el]
                eas[pos:pos + ne] = ea[el]
                ib[bpos:bpos + ne] = srcp[el] - half
                pos += K_B[b] * 128
                bpos += K_B[b] * 128
        assert pos == S * 128 and apos == SA * 128 and bpos == SB * 128

        def wrap(a):
            return np.ascontiguousarray(a.reshape(-1, 128).T)

        per_core.append(dict(
            idx_a=tile16(ia.astype(np.int16)),
            idx_b=tile16(ib.astype(np.int16)),
            dlocw=wrap(dlocs).astype(ml_dtypes.bfloat16),
            eat16=np.ascontiguousarray(eas.T).astype(ml_dtypes.bfloat16)))

    # shared tensors
    xt = np.zeros((IN, nt), np.float32)
    for c in range(NCORES):
        xt[:, c * npad + perm[c]] = x[c * npc:(c + 1) * npc].T
    bf = ml_dtypes.bfloat16
    shared = dict(
        xt_full=xt.astype(bf),
        w_kv=np.concatenate([inputs["tk_w"], inputs["tv_w"]],
                            axis=1).astype(bf),
        w_q=np.asarray(inputs["tq_w"]).astype(bf),
        w_te=np.asarray(inputs["te_w"]).astype(bf),
        w_ge=np.asarray(inputs["ge_w"]).astype(bf),
        w_sk=np.asarray(inputs["ts_w"]).astype(bf),
        w_l0=np.asarray(inputs["lin0_w"]).astype(bf),
        w_g1=np.concatenate([inputs["g1_wi"], inputs["g1_wh"]],
                            axis=1).astype(bf),
        w_lin1=np.asarray(inputs["lin1_w"]).astype(bf),
        w_gl=np.asarray(inputs["gl_w"]).astype(bf),
        w_gr=np.asarray(inputs["gr_w"]).astype(bf),
        w_g2=np.concatenate([inputs["g2_wi"], inputs["g2_wh"]],
                            axis=1).astype(bf),
        iota_in=np.tile(np.arange(128, dtype=np.float32), (128, 1)).astype(bf),
        gatt_b=np.tile(np.asarray(inputs["g_att"], np.float32), (128, 1)),
        kvb_b=np.tile(np.concatenate([inputs["tk_b"], inputs["tv_b"]]),
                      (128, 2)).astype(np.float32),
        tqb_b=np.tile(np.asarray(inputs["tq_b"], np.float32), (128, 2)),
    )
    bias = np.zeros((128, NB), np.float32)

    def put(j, v):
        v = np.asarray(v, np.float32)
        bias[:len(v), j] = v

    put(B_TS, inputs["ts_b"])
    put(B_L0, inputs["lin0_b"])
    g1bi, g1bh = np.asarray(inputs["g1_bi"]), np.asarray(inputs["g1_bh"])
    put(B_G1R, g1bi[0:128] + g1bh[0:128])
    put(B_G1Z, g1bi[128:256] + g1bh[128:256])
    put(B_G1IN, g1bi[256:384])
    put(B_G1HN, g1bh[256:384])
    put(B_LIN1, inputs["lin1_b"])
    put(B_GL, inputs["gl_b"])
    put(B_GR, inputs["gr_b"])
    put(B_GB, inputs["g_bias"])
    g2bi, g2bh = np.asarray(inputs["g2_bi"]), np.asarray(inputs["g2_bh"])
    put(B_G2R, g2bi[0:64] + g2bh[0:64])
    put(B_G2Z, g2bi[64:128] + g2bh[64:128])
    put(B_G2IN, g2bi[128:192])
    put(B_G2HN, g2bh[128:192])
    put(B_IOTA, np.arange(128, dtype=np.float32))
    shared["biases"] = bias

    in_maps = []
    for c in range(NCORES):
        m = dict(shared)
        m["xt_own"] = np.ascontiguousarray(
            xt[:, c * npad:(c + 1) * npad]).astype(bf)
        m.update(per_core[c])
        in_maps.append(m)
    cfg = dict(npc=npc, npad=npad, K_A=K_A, K_B=K_B, perm=perm,
               ch=448 if npad % 448 == 0 else 128)
    return in_maps, cfg


def run(inputs, npc, npad, trace=False):
    in_maps, cfg = prep_inputs(inputs, npc, npad)
    nc = build_program(cfg)
    from concourse.bass_utils import run_bass_kernel_spmd
    res = run_bass_kernel_spmd(nc, in_maps, list(range(NCORES)), trace=trace)
    outs = []
    for c in range(NCORES):
        o = res.results[c]["out"]          # [HID, npad]
        outs.append(o[:, cfg["perm"][c]].T)
    full = np.concatenate(outs, axis=0).astype(np.float32)
    return full, res


def kernel(**inputs) -> np.ndarray:
    out, _ = run(inputs, npc=6250, npad=6272)
    return out

